# revision 24
# baseline (speedup 1.0000x reference)
"""Trainium2 Bass kernel for a DeepseekV2 decoder-layer attention block
(MLA prefill, fp32 reference) distributed across 8 NeuronCores.

Strategy (single NEFF, SPMD on 8 cores):
  - Sequence-shard the shared projections: each core computes ckv / k_pe
    (RMS-normed / roped) then q_lora for its 256 rows of the sequence, in
    transposed layout; two on-device AllGathers replicate them. ckv goes
    first so its AllGather flies under the q_lora GEMM; the q_lora
    AllGather flies under the K/V expansion.
  - Head-shard the rest (4 heads per core): q_b projection + RoPE, kc/vc
    expansion, causal attention (scores computed transposed so the attn@v
    matmul needs no transposes), and a row-shard of w_o.
  - Each core emits a partial [S, HID] bf16 output; the host sums the 8
    partials (the output all-reduce) to produce the full result.

Perf structure (v3):
  - All weight/activation streams are host-packed into [128, N] layouts so
    every DMA is one large transfer, spread round-robin across the
    sync/scalar/gpsimd/vector DGE rings (the per-DMA ~600ns issue cost made
    the v2 front end ring-bound).
  - Attention is software-pipelined (lookahead 2); the causal mask is
    applied as a third matmul into the score PSUM group (-1e30 * step),
    so the exp -> attn@v chain has no vector-engine hop.
  - Softmax denominators: bf16 P-tile running sum on the vector engine,
    one ones-matmul per (head, chunk), reciprocal_approx_fast.
  - RoPE rotation via partition-offset vector ops with sign-folded sin.
  - w_o resident; output staged bf16 through the scalar engine.
"""

import numpy as np

S, HID, H = 2048, 5120, 32
QLR, KVLR = 1536, 512
DN, DR, DV = 128, 64, 128
DQ = DN + DR
NC_N = 8
HPC = H // NC_N          # heads per core
SL = S // NC_N           # sequence rows per core (front end)
ROPE_BASE, EPS = 10000.0, 1e-6

_CACHE = {}


def _bf16():
    import ml_dtypes
    return np.dtype(ml_dtypes.bfloat16)


def _build_program():
    import concourse.bass as bass
    import concourse.tile as tile
    from concourse import bacc, mybir
    from contextlib import ExitStack

    f32 = mybir.dt.float32
    bf = mybir.dt.bfloat16
    AF = mybir.ActivationFunctionType

    nc = bacc.Bacc("TRN2", target_bir_lowering=False, debug=False,
                   num_devices=NC_N)

    def din(name, shape, dt=bf):
        return nc.dram_tensor(name, list(shape), dt, kind="ExternalInput").ap()

    KH = HID // 128       # 40 k-chunks of the model dim
    K2 = KH // 2          # 20 double-chunks
    KQ = QLR // 128       # 12 chunks of the q-lora dim
    KC = KVLR // 128      # 4 chunks of the kv-lora dim
    SC = S // 512         # 4 sequence chunks of 512
    SB = S // 128         # 16 sequence blocks of 128
    NW = HID // 512       # 10 w_o column chunks

    hsT2_d = din("hsT2", (128, K2 * 2 * SL))    # [p, k2*512+half*256+c]
    wqa_d = din("wqa", (HID, QLR))
    wkvpe2_d = din("wkvpe2", (128, (HID // 256) * 1280))  # packed pairs
    cosl_d = din("cosl", (128, SL))
    sinlsg_d = din("sinlsg", (128, SL))         # sign-folded sin
    cosf_d = din("cosf", (128, S))
    sinfsg_d = din("sinfsg", (128, S))
    wqb2_d = din("wqb2", (128, KQ * 768))       # [p, k*768 + col]
    kct2_d = din("kct2", (128, HPC * KC * 128))  # [p, (i*4+c)*128 + d]
    vcp2_d = din("vcp2", (128, KC * HPC * DV))  # [p, c*512 + col]
    wo2_d = din("wo2", (128, HPC * NW * 512))   # [p, (i*NW+n)*512 + col]
    steps_d = din("steps", (128, 4 * 512))      # [r, p*512+q] = [p*128+r > q]
    negeye_d = din("negeye", (128, 128))        # -1e30 * I
    out_d = nc.dram_tensor("out_partial", [S, HID], bf,
                           kind="ExternalOutput").ap()

    cc1_in = nc.dram_tensor("cc1_in", [KVLR + 128, SL], bf).ap()
    cc1_out = nc.dram_tensor("cc1_out", [NC_N * (KVLR + 128), SL], bf,
                             addr_space="Shared").ap()
    cc2_in = nc.dram_tensor("cc2_in", [QLR, SL], bf).ap()
    cc2_out = nc.dram_tensor("cc2_out", [NC_N * QLR, SL], bf,
                             addr_space="Shared").ap()

    with tile.TileContext(nc) as tc, ExitStack() as ctx:
        def pool(name, bufs):
            return ctx.enter_context(tc.tile_pool(name=name, bufs=bufs))

        p_hs = pool("hs", 4)
        p_w = pool("wstr", 3)
        p_raw = pool("raw", 12)
        p_sq = pool("sqt", 2)
        p_scn = pool("scn", 2)
        p_sml = pool("sml", 4)
        p_one = pool("ones", 2)
        p_cs = pool("cs", 2)
        p_csl = pool("csl", 2)
        p_bc = pool("bc", 2)
        p_kc = pool("kc", 1)
        p_vc = pool("vc", 1)
        p_wqb = pool("wqb", 2)
        p_wo = pool("wo", 5)
        p_msk = pool("msk", 1)
        p_kpeg = pool("kpeg", 4)
        p_K = pool("Kt", 4)
        p_V = pool("Vt", 16)
        p_qn = pool("Qn", 4)
        p_rope = pool("rope", 2)
        p_f32 = pool("fr32", 2)
        p_ckvg = pool("ckvg", 16)
        p_wk = pool("wk", 14)       # shared ring: qlg -> P tiles
        p_oT = pool("oT", 4)
        p_out = pool("outst", 3)

        pp_o = ctx.enter_context(
            tc.tile_pool(name="ppo", bufs=4, space="PSUM"))
        pp_s = ctx.enter_context(
            tc.tile_pool(name="pps", bufs=3, space="PSUM"))
        pp_sm = ctx.enter_context(
            tc.tile_pool(name="psm", bufs=1, space="PSUM"))

        ones_col = p_one.tile([128, 1], bf)       # lhsT for column sums
        nc.vector.memset(ones_col[:], 1.0)
        eps_t = p_one.tile([1, 1], f32, tag="eps", name="eps")
        nc.vector.memset(eps_t[:], EPS)

        def rope_apply(dst, src_ps, cos_t, sin_t, c0, width, u1, u2):
            """dst(bf16) = src*cos + rot(src)*sin_signed, via partition-offset
            muls. src_ps is a [128, width] f32 PSUM pack of 64-dim halves."""
            nc.vector.tensor_mul(u1[:, :width], src_ps[:],
                                 cos_t[:, c0:c0 + width])
            for blk in (0, 64):
                nc.vector.tensor_mul(
                    u2[blk:blk + 32, :width],
                    src_ps[blk + 32:blk + 64, :],
                    sin_t[blk:blk + 32, c0:c0 + width])
                nc.vector.tensor_mul(
                    u2[blk + 32:blk + 64, :width],
                    src_ps[blk:blk + 32, :],
                    sin_t[blk + 32:blk + 64, c0:c0 + width])
            nc.vector.tensor_add(dst, u1[:, :width], u2[:, :width])

        # ---------------- FRONT 1: ckv + k_pe pass (then AG1) --------------
        ckv_ps = [pp_o.tile([128, SL], f32, tag="po", name="po")
                  for _ in range(KC)]
        pe_ps = pp_s.tile([128, SL], f32, tag="ps", name="ps")
        for k2 in range(K2):
            hs2 = p_hs.tile([128, 2 * SL], bf, tag="hs", name="hs")
            nc.sync.dma_start(hs2[:], hsT2_d[:, k2 * 512:(k2 + 1) * 512])
            wv = p_w.tile([128, 1280], bf, tag="wkv", name="wkv", bufs=3)
            nc.scalar.dma_start(wv[:], wkvpe2_d[:, k2 * 1280:(k2 + 1) * 1280])
            for kk in range(2):
                k = 2 * k2 + kk
                hh = hs2[:, kk * SL:(kk + 1) * SL]
                wk0 = kk * 640
                for c in range(KC):
                    nc.tensor.matmul(ckv_ps[c][:],
                                     wv[:, wk0 + c * 128:wk0 + (c + 1) * 128],
                                     hh, start=(k == 0), stop=(k == KH - 1))
                nc.tensor.matmul(pe_ps[:], wv[:, wk0 + KVLR:wk0 + KVLR + 128],
                                 hh,
                                 start=(k == 0), stop=(k == KH - 1))
        ssq_kv = pp_sm.tile([1, SL], f32, tag="sm", name="sm")
        raw_kv = []
        for c in range(KC):
            r = p_raw.tile([128, SL], bf, tag="raw", name="raw")
            nc.scalar.activation(r[:], ckv_ps[c][:], AF.Copy)
            raw_kv.append(r)
            sq = p_sq.tile([128, SL], bf, tag="sq", name="sq")
            nc.scalar.activation(sq[:], ckv_ps[c][:], AF.Square)
            nc.tensor.matmul(ssq_kv[:], ones_col[:], sq[:],
                             start=(c == 0), stop=(c == KC - 1))
        t_kv = p_sml.tile([1, SL], f32, tag="sml", name="sml")
        nc.scalar.activation(t_kv[:], ssq_kv[:], AF.Sqrt,
                             bias=eps_t[:], scale=1.0 / KVLR)
        s_kv = p_sml.tile([1, SL], f32, tag="sml", name="sml")
        nc.vector.reciprocal(s_kv[:], t_kv[:])
        bkv_sb = p_bc.tile([128, 512], f32, tag="bc", name="bc")
        nc.gpsimd.partition_broadcast(bkv_sb[:, :SL], s_kv[:])
        for c in range(KC):
            cn = p_scn.tile([128, SL], bf, tag="scn", name="scn")
            nc.vector.tensor_mul(cn[:], raw_kv[c][:], bkv_sb[:, :SL])
            nc.scalar.dma_start(cc1_in[c * 128:(c + 1) * 128, :], cn[:])
        # rope k_pe (partition-offset rotation, sign folded into sin table)
        cosl_t = p_csl.tile([128, SL], bf, tag="csl", name="csl")
        sinl_t = p_csl.tile([128, SL], bf, tag="csl", name="csl")
        nc.sync.dma_start(cosl_t[:], cosl_d[:, :])
        nc.sync.dma_start(sinl_t[:], sinlsg_d[:, :])
        u1f = p_f32.tile([128, 512], f32, tag="f32", name="f32")
        u2f = p_f32.tile([128, 512], f32, tag="f32", name="f32")
        kpe_n = p_scn.tile([128, SL], bf, tag="scn", name="scn")
        rope_apply(kpe_n[:], pe_ps, cosl_t, sinl_t, 0, SL, u1f, u2f)
        nc.scalar.dma_start(cc1_in[KVLR:KVLR + 128, :], kpe_n[:])

        nc.gpsimd.collective_compute(
            "AllGather", mybir.AluOpType.bypass,
            ins=[cc1_in[:]], outs=[cc1_out[:]],
            replica_groups=[list(range(NC_N))],
        )

        # ---------------- FRONT 2: q_lora pass (then AG2) ------------------
        # resident back-end weights stream on the scalar/gpsimd rings
        kct_t = p_kc.tile([128, HPC * KC * 128], bf, tag="kc", name="kc")
        nc.scalar.dma_start(kct_t[:], kct2_d[:, :])
        vcp_t = p_vc.tile([128, KC * HPC * DV], bf, tag="vc", name="vc")
        nc.gpsimd.dma_start(vcp_t[:], vcp2_d[:, :])
        wqb_t = []
        for h in range(2):
            t = p_wqb.tile([128, 6 * 768], bf, tag="wqb", name="wqb")
            eng = nc.scalar if h == 0 else nc.gpsimd
            eng.dma_start(t[:], wqb2_d[:, h * 6 * 768:(h + 1) * 6 * 768])
            wqb_t.append(t)
        cosf_t = p_cs.tile([128, S], bf, tag="cs", name="cs")
        sinf_t = p_cs.tile([128, S], bf, tag="cs", name="cs")
        nc.scalar.dma_start(cosf_t[:], cosf_d[:, :])
        nc.gpsimd.dma_start(sinf_t[:], sinfsg_d[:, :])
        steps_t = p_msk.tile([128, 4 * 512], bf, tag="msk", name="msk")
        nc.scalar.dma_start(steps_t[:], steps_d[:, :])
        negeye_t = p_msk.tile([128, 128], bf, tag="ne", name="ne")
        nc.gpsimd.dma_start(negeye_t[:], negeye_d[:, :])

        def wqb_ap(k, c0, c1):
            return wqb_t[k // 6][:, (k % 6) * 768 + c0:(k % 6) * 768 + c1]

        ssq_q = pp_sm.tile([1, SL], f32, tag="sm", name="sm")
        raw_q = []
        for g in range(2):
            ql_ps = ([pp_o.tile([128, SL], f32, tag="po", name="po")
                      for _ in range(4)] +
                     [pp_s.tile([128, SL], f32, tag="ps", name="ps")
                      for _ in range(2)])
            for k2 in range(K2):
                hs2 = p_hs.tile([128, 2 * SL], bf, tag="hs", name="hs")
                heng = nc.sync if k2 % 2 == 0 else nc.scalar
                heng.dma_start(hs2[:], hsT2_d[:, k2 * 512:(k2 + 1) * 512])
                wq = []
                for kk in range(2):
                    k = 2 * k2 + kk
                    w = p_w.tile([128, QLR // 2], bf, tag="wqa", name="wqa",
                                 bufs=6)
                    eng = nc.sync if kk == 0 else nc.scalar
                    eng.dma_start(
                        w[:], wqa_d[k * 128:(k + 1) * 128,
                                    g * (QLR // 2):(g + 1) * (QLR // 2)])
                    wq.append(w)
                for kk in range(2):
                    k = 2 * k2 + kk
                    hh = hs2[:, kk * SL:(kk + 1) * SL]
                    for mi in range(6):
                        nc.tensor.matmul(ql_ps[mi][:],
                                         wq[kk][:, mi * 128:(mi + 1) * 128],
                                         hh, start=(k == 0),
                                         stop=(k == KH - 1))
            for mi in range(6):
                m = g * 6 + mi
                r = p_raw.tile([128, SL], bf, tag="raw", name="raw")
                nc.scalar.activation(r[:], ql_ps[mi][:], AF.Copy)
                raw_q.append(r)
                sq = p_sq.tile([128, SL], bf, tag="sq", name="sq")
                nc.scalar.activation(sq[:], ql_ps[mi][:], AF.Square)
                nc.tensor.matmul(ssq_q[:], ones_col[:], sq[:],
                                 start=(m == 0), stop=(m == KQ - 1))
        t_q = p_sml.tile([1, SL], f32, tag="sml", name="sml")
        nc.scalar.activation(t_q[:], ssq_q[:], AF.Sqrt,
                             bias=eps_t[:], scale=1.0 / QLR)
        s_q = p_sml.tile([1, SL], f32, tag="sml", name="sml")
        nc.vector.reciprocal(s_q[:], t_q[:])
        bq_sb = p_bc.tile([128, 512], f32, tag="bc", name="bc")
        nc.gpsimd.partition_broadcast(bq_sb[:, :SL], s_q[:])
        for m in range(KQ):
            qn = p_scn.tile([128, SL], bf, tag="scn", name="scn")
            nc.vector.tensor_mul(qn[:], raw_q[m][:], bq_sb[:, :SL])
            nc.scalar.dma_start(cc2_in[m * 128:(m + 1) * 128, :], qn[:])

        nc.gpsimd.collective_compute(
            "AllGather", mybir.AluOpType.bypass,
            ins=[cc2_in[:]], outs=[cc2_out[:]],
            replica_groups=[list(range(NC_N))],
        )

        # ---------------- BACK: K/V expansion (needs AG1 only) -------------
        RPC = 512 // SL     # AG rank-blocks per 512-wide seq chunk
        g_engs = [nc.scalar, nc.gpsimd, nc.sync]
        g_rr = [0]

        def rr_eng():
            e = g_engs[g_rr[0] % 3]
            g_rr[0] += 1
            return e

        ckvg = {}
        for c in range(KC):
            for sc in range(SC):
                t = p_ckvg.tile([128, 512], bf, tag="ckvg", name="ckvg")
                for half in range(RPC):
                    r = RPC * sc + half
                    rr_eng().dma_start(
                        t[:, half * SL:(half + 1) * SL],
                        cc1_out[r * (KVLR + 128) + c * 128:
                                r * (KVLR + 128) + (c + 1) * 128, :])
                ckvg[(c, sc)] = t
        kpeg = {}
        for sc in range(SC):
            t = p_kpeg.tile([128, 512], bf, tag="kpeg", name="kpeg")
            for half in range(RPC):
                r = RPC * sc + half
                rr_eng().dma_start(
                    t[:, half * SL:(half + 1) * SL],
                    cc1_out[r * (KVLR + 128) + KVLR:
                            r * (KVLR + 128) + KVLR + 128, :])
            kpeg[sc] = t
        # w_o resident tiles: streamed after the gathers
        wo_t = []
        for h in range(5):
            t = p_wo.tile([128, 8 * 512], bf, tag="wo", name="wo")
            eng = g_engs[h % 2]
            eng.dma_start(t[:], wo2_d[:, h * 8 * 512:(h + 1) * 8 * 512])
            wo_t.append(t)

        def wo_ap(i, n):
            b = i * NW + n
            return wo_t[b // 8][:, (b % 8) * 512:(b % 8 + 1) * 512]

        # K^T per head: [DN, S]
        K_t = []
        for i in range(HPC):
            kt = p_K.tile([128, S], bf, tag="K", name="K")
            K_t.append(kt)
            for sc in range(SC):
                ps = pp_s.tile([128, 512], f32, tag="ps", name="ps")
                for c in range(KC):
                    nc.tensor.matmul(
                        ps[:],
                        kct_t[:, (i * KC + c) * 128:(i * KC + c + 1) * 128],
                        ckvg[(c, sc)][:],
                        start=(c == 0), stop=(c == KC - 1))
                nc.scalar.activation(kt[:, sc * 512:(sc + 1) * 512], ps[:],
                                     AF.Copy)

        # V natural: per seq-block [128, 4*DV]
        V_t = []
        for sb in range(SB):
            ps = pp_s.tile([128, 512], f32, tag="ps", name="ps")
            for c in range(KC):
                nc.tensor.matmul(
                    ps[:],
                    ckvg[(c, sb // 4)][:, (sb % 4) * 128:(sb % 4 + 1) * 128],
                    vcp_t[:, c * 512:(c + 1) * 512],
                    start=(c == 0), stop=(c == KC - 1))
            vt = p_V.tile([128, HPC * DV], bf, tag="V", name="V")
            nc.scalar.activation(vt[:], ps[:], AF.Copy)
            V_t.append(vt)

        # ---------------- BACK: per-chunk Q proj + attention + w_o ---------
        qg_engs = [nc.gpsimd, nc.scalar]

        def gather_qlg(sc):
            qlg = []
            for k in range(KQ):
                t = p_wk.tile([128, 512], bf, tag="wk", name="wk")
                for half in range(RPC):
                    r = RPC * sc + half
                    qg_engs[(2 * k + half) % 2].dma_start(
                        t[:, half * SL:(half + 1) * SL],
                        cc2_out[r * QLR + k * 128:r * QLR + (k + 1) * 128, :])
                qlg.append(t)
            return qlg

        qlg = gather_qlg(0)
        for sc in range(SC):
            # --- Q^T nope per head + pe packs (roped) ---
            qn_t = []
            for i in range(HPC):
                ps = pp_o.tile([128, 512], f32, tag="po", name="po")
                for k in range(KQ):
                    nc.tensor.matmul(ps[:], wqb_ap(k, i * 128, (i + 1) * 128),
                                     qlg[k][:], start=(k == 0),
                                     stop=(k == KQ - 1))
                qt = p_qn.tile([128, 512], bf, tag="Qn", name="Qn")
                nc.scalar.activation(qt[:], ps[:], AF.Copy)
                qn_t.append(qt)
            roped = []
            for pkt in range(2):
                ps_pe = pp_s.tile([128, 512], f32, tag="ps", name="ps")
                for k in range(KQ):
                    nc.tensor.matmul(
                        ps_pe[:],
                        wqb_ap(k, 512 + pkt * 128, 512 + (pkt + 1) * 128),
                        qlg[k][:], start=(k == 0), stop=(k == KQ - 1))
                u1 = p_f32.tile([128, 512], f32, tag="f32", name="f32")
                u2 = p_f32.tile([128, 512], f32, tag="f32", name="f32")
                rp = p_rope.tile([128, 512], bf, tag="rope", name="rope")
                rope_apply(rp[:], ps_pe, cosf_t, sinf_t, sc * 512, 512, u1, u2)
                roped.append(rp)

            # --- attention, software-pipelined over j (lookahead 2) ---
            o_ps_l, dinv_l = [], []
            for i in range(HPC):
                pkt, hp = i // 2, i % 2
                o_ps = pp_o.tile([128, 512], f32, tag="po", name="po")
                o_ps_l.append(o_ps)
                d_ps = pp_sm.tile([1, 512], f32, tag="sm", name="sm")
                nj = 4 * sc + 4
                pts = []

                def consume(j):
                    nc.tensor.matmul(o_ps[:],
                                     V_t[j][:, i * DV:(i + 1) * DV],
                                     pts[j][:], start=(j == 0),
                                     stop=(j == nj - 1))
                    nc.tensor.matmul(d_ps[:], ones_col[:], pts[j][:],
                                     start=(j == 0), stop=(j == nj - 1))

                for j in range(nj):
                    s_ps = pp_s.tile([128, 512], f32, tag="ps", name="ps")
                    nc.tensor.matmul(s_ps[:],
                                     K_t[i][:, j * 128:(j + 1) * 128],
                                     qn_t[i][:], start=True, stop=False)
                    if j >= 4 * sc:
                        p = j - 4 * sc
                        nc.tensor.matmul(
                            s_ps[:], negeye_t[:],
                            steps_t[:, p * 512:(p + 1) * 512],
                            start=False, stop=False)
                    nc.tensor.matmul(
                        s_ps[:],
                        kpeg[j // 4][hp * 64:(hp + 1) * 64,
                                     (j % 4) * 128:(j % 4 + 1) * 128],
                        roped[pkt][hp * 64:(hp + 1) * 64, :],
                        start=False, stop=True)
                    pt = p_wk.tile([128, 512], bf, tag="wk", name="wk")
                    nc.scalar.activation(pt[:], s_ps[:], AF.Exp)
                    pts.append(pt)
                    if j >= 2:
                        consume(j - 2)
                for j in range(max(0, nj - 2), nj):
                    consume(j)
                dinv = p_sml.tile([1, 512], f32, tag="sml", name="sml")
                nc.vector.reciprocal_approx_fast(dinv[:], d_ps[:])
                dinv_l.append(dinv)

            oT = []
            for i in range(HPC):
                bc_sb = p_bc.tile([128, 512], f32, tag="bc", name="bc")
                nc.gpsimd.partition_broadcast(bc_sb[:], dinv_l[i][:])
                ot = p_oT.tile([128, 512], bf, tag="oT", name="oT")
                nc.vector.tensor_mul(ot[:], o_ps_l[i][:], bc_sb[:])
                oT.append(ot)

            # prefetch next chunk's gathered q_lora during w_o
            if sc + 1 < SC:
                qlg = gather_qlg(sc + 1)

            # --- w_o partial for this seq chunk ---
            # sbl outer / paired n inner: two [128,512] psum groups stage
            # into one [128,1024] tile and ship with a single DMA
            for sbl in range(4):
                sb = sc * 4 + sbl
                for n2 in range(NW // 2):
                    ob = p_out.tile([128, 1024], bf, tag="outst",
                                    name="outst")
                    for half in range(2):
                        n = 2 * n2 + half
                        ps = pp_s.tile([128, 512], f32, tag="ps", name="ps")
                        for i in range(HPC):
                            nc.tensor.matmul(
                                ps[:], oT[i][:, sbl * 128:(sbl + 1) * 128],
                                wo_ap(i, n), start=(i == 0),
                                stop=(i == HPC - 1))
                        ceng = nc.scalar if half == 0 else nc.vector
                        if half == 0:
                            ceng.activation(ob[:, :512], ps[:], AF.Copy)
                        else:
                            ceng.tensor_copy(ob[:, 512:], ps[:])
                    nc.sync.dma_start(
                        out_d[sb * 128:(sb + 1) * 128,
                              n2 * 1024:(n2 + 1) * 1024], ob[:])

    nc.compile()
    return nc


def _prep_inputs(inputs):
    """Host-side sharding + weight folding. Returns in_maps (list of 8 dicts)."""
    BF = _bf16()

    hs = np.asarray(inputs['hidden_states'], np.float32)
    pos = np.asarray(inputs['positions'])
    w_qa = np.asarray(inputs['w_qa'], np.float32)
    q_a_ln_w = np.asarray(inputs['q_a_ln_w'], np.float32)
    w_qb = np.asarray(inputs['w_qb'], np.float32)
    w_kva = np.asarray(inputs['w_kva'], np.float32)
    kv_a_ln_w = np.asarray(inputs['kv_a_ln_w'], np.float32)
    kc = np.asarray(inputs['kc'], np.float32)
    vc = np.asarray(inputs['vc'], np.float32)
    w_o = np.asarray(inputs['w_o'], np.float32)

    perm = np.concatenate([np.arange(0, DR, 2), np.arange(1, DR, 2)])
    inv_freq = 1.0 / (ROPE_BASE ** (np.arange(0, DR, 2, dtype=np.float64) / DR))
    freqs = pos.astype(np.float64)[None, :] * inv_freq[:, None]     # [32, S]
    cosT = np.cos(freqs).astype(np.float32)
    sinT = np.sin(freqs).astype(np.float32)
    cos128 = np.tile(cosT, (4, 1)).astype(BF)                        # [128, S]
    sin128 = np.tile(sinT, (4, 1)).astype(np.float32)
    sgn = np.where((np.arange(128) % 64) < 32, -1.0, 1.0)[:, None]
    sinsg128 = (sin128 * sgn).astype(BF)

    scale = DQ ** -0.5
    w_qb_eff = ((w_qb * q_a_ln_w[:, None]) * scale).reshape(QLR, H, DQ)

    w_pe = w_kva[:, KVLR:][:, perm]
    wkvpe = np.concatenate([w_kva[:, :KVLR], w_pe, w_pe], 1)   # [HID, 640]
    K2h = (HID // 128) // 2
    wkvpe2 = wkvpe.reshape(K2h, 2, 128, 640).transpose(2, 0, 1, 3) \
        .reshape(128, K2h * 1280).astype(BF)

    kc_f = kc * kv_a_ln_w[None, None, :]
    vc_f = vc * kv_a_ln_w[None, :, None]

    # step tables: steps[r, p*512+q] = 1 if p*128+r > q else 0
    steps = np.zeros((128, 4 * 512), np.float32)
    rr = np.arange(128)[:, None]
    qq = np.arange(512)[None, :]
    for p in range(4):
        steps[:, p * 512:(p + 1) * 512] = (p * 128 + rr > qq)
    steps_b = steps.astype(BF)
    negeye = (-1e30 * np.eye(128, dtype=np.float32)).astype(BF)

    wqa_b = w_qa.astype(BF)

    K2 = (HID // 128) // 2
    NW = HID // 512

    in_maps = []
    for core in range(NC_N):
        rows = slice(core * SL, (core + 1) * SL)
        h0 = core * HPC

        hsT = np.ascontiguousarray(hs[rows].T)                   # [HID, SL]
        hsT2 = hsT.reshape(K2, 2, 128, SL).transpose(2, 0, 1, 3) \
            .reshape(128, K2 * 2 * SL)

        wqb_all = np.empty((QLR, 768), np.float32)
        for i in range(HPC):
            wqb_all[:, i * 128:(i + 1) * 128] = w_qb_eff[:, h0 + i, :DN]
        for pkt in range(2):
            a, b = h0 + 2 * pkt, h0 + 2 * pkt + 1
            pe_a = w_qb_eff[:, a, DN:][:, perm]
            pe_b = w_qb_eff[:, b, DN:][:, perm]
            wqb_all[:, 512 + pkt * 128:512 + pkt * 128 + 64] = pe_a
            wqb_all[:, 512 + pkt * 128 + 64:512 + (pkt + 1) * 128] = pe_b
        wqb2 = wqb_all.reshape(12, 128, 768).transpose(1, 0, 2) \
            .reshape(128, 12 * 768)

        kct = np.stack([kc_f[h0 + i].T[c * 128:(c + 1) * 128]
                        for i in range(HPC) for c in range(KVLR // 128)])
        kct2 = kct.transpose(1, 0, 2).reshape(128, -1)           # [128, 2048]

        vcp = np.concatenate([vc_f[h0 + i] for i in range(HPC)], 1)
        vcp2 = vcp.reshape(KVLR // 128, 128, HPC * DV) \
            .transpose(1, 0, 2).reshape(128, -1)                 # [128, 2048]

        wo_sh = w_o[h0 * DV:(h0 + HPC) * DV, :]                  # [512, HID]
        wo2 = wo_sh.reshape(HPC, 128, NW, 512).transpose(0, 2, 1, 3) \
            .reshape(HPC * NW, 128, 512).transpose(1, 0, 2) \
            .reshape(128, -1)                                    # [128, 20480]

        in_maps.append({
            "hsT2": hsT2.astype(BF),
            "wqa": wqa_b,
            "wkvpe2": wkvpe2,
            "cosl": np.ascontiguousarray(cos128[:, rows]),
            "sinlsg": np.ascontiguousarray(sinsg128[:, rows]),
            "cosf": cos128,
            "sinfsg": sinsg128,
            "wqb2": wqb2.astype(BF),
            "kct2": kct2.astype(BF),
            "vcp2": vcp2.astype(BF),
            "wo2": wo2.astype(BF),
            "steps": steps_b,
            "negeye": negeye,
        })
    return in_maps


def _get_program():
    if "nc" not in _CACHE:
        _CACHE["nc"] = _build_program()
    return _CACHE["nc"]


def run(inputs, trace=False, trace_kwargs=None):
    """Build (cached), run on 8 cores, return (output, BassKernelResults)."""
    from concourse.bass_utils import run_bass_kernel_spmd

    nc = _get_program()
    in_maps = _prep_inputs(inputs)
    res = run_bass_kernel_spmd(nc, in_maps, list(range(NC_N)),
                               trace=trace, **(trace_kwargs or {}))
    out = np.zeros((S, HID), np.float32)
    for r in res.results:
        out += np.asarray(r["out_partial"], dtype=np.float32)
    return out, res


def kernel(**inputs) -> np.ndarray:
    out, _ = run(inputs, trace=False)
    return out


# revision 27
# speedup vs baseline: 1.0973x; 1.0973x over previous
"""Trainium2 Bass kernel for a DeepseekV2 decoder-layer attention block
(MLA prefill, fp32 reference) distributed across 8 NeuronCores.

Strategy (single NEFF, SPMD on 8 cores):
  - Sequence-shard the shared projections: each core computes ckv / k_pe
    (RMS-normed / roped) then q_lora for its 256 rows of the sequence, in
    transposed layout; two on-device AllGathers replicate them. ckv goes
    first so its AllGather flies under the q_lora GEMM; the q_lora
    AllGather flies under the K/V expansion.
  - Head-shard the rest (4 heads per core): q_b projection + RoPE, kc/vc
    expansion, causal attention (scores computed transposed so the attn@v
    matmul needs no transposes), and a row-shard of w_o.
  - Each core emits a partial [S, HID] bf16 output; the host sums the 8
    partials (the output all-reduce) to produce the full result.

Perf structure (v3):
  - All weight/activation streams are host-packed into [128, N] layouts so
    every DMA is one large transfer, spread round-robin across the
    sync/scalar/gpsimd/vector DGE rings (the per-DMA ~600ns issue cost made
    the v2 front end ring-bound).
  - Attention is software-pipelined (lookahead 2); the causal mask is
    applied as a third matmul into the score PSUM group (-1e30 * step),
    so the exp -> attn@v chain has no vector-engine hop.
  - Softmax denominators: bf16 P-tile running sum on the vector engine,
    one ones-matmul per (head, chunk), reciprocal_approx_fast.
  - RoPE rotation via partition-offset vector ops with sign-folded sin.
  - w_o resident; output staged bf16 through the scalar engine.
"""

import numpy as np

S, HID, H = 2048, 5120, 32
QLR, KVLR = 1536, 512
DN, DR, DV = 128, 64, 128
DQ = DN + DR
NC_N = 8
HPC = H // NC_N          # heads per core
SL = S // NC_N           # sequence rows per core (front end)
ROPE_BASE, EPS = 10000.0, 1e-6

_CACHE = {}


def _bf16():
    import ml_dtypes
    return np.dtype(ml_dtypes.bfloat16)


def _build_program():
    import concourse.bass as bass
    import concourse.tile as tile
    from concourse import bacc, mybir
    from contextlib import ExitStack

    f32 = mybir.dt.float32
    bf = mybir.dt.bfloat16
    AF = mybir.ActivationFunctionType

    nc = bacc.Bacc("TRN2", target_bir_lowering=False, debug=False,
                   num_devices=NC_N)

    def din(name, shape, dt=bf):
        return nc.dram_tensor(name, list(shape), dt, kind="ExternalInput").ap()

    KH = HID // 128       # 40 k-chunks of the model dim
    K2 = KH // 2          # 20 double-chunks
    KQ = QLR // 128       # 12 chunks of the q-lora dim
    KC = KVLR // 128      # 4 chunks of the kv-lora dim
    SC = S // 512         # 4 sequence chunks of 512
    SB = S // 128         # 16 sequence blocks of 128
    NW = HID // 512       # 10 w_o column chunks

    hsT2_d = din("hsT2", (128, K2 * 2 * SL))    # [p, k2*512+half*256+c]
    wqa_d = din("wqa", (HID, QLR))
    wkvpe2_d = din("wkvpe2", (128, (HID // 256) * 1280))  # packed pairs
    cosl_d = din("cosl", (128, SL))
    sinlsg_d = din("sinlsg", (128, SL))         # sign-folded sin
    cosf_d = din("cosf", (128, S))
    sinfsg_d = din("sinfsg", (128, S))
    wqb2_d = din("wqb2", (128, KQ * 768))       # [p, k*768 + col]
    kct2_d = din("kct2", (128, HPC * KC * 128))  # [p, (i*4+c)*128 + d]
    vcp2_d = din("vcp2", (128, KC * HPC * DV))  # [p, c*512 + col]
    wo2_d = din("wo2", (128, HPC * NW * 512))   # [p, (i*NW+n)*512 + col]
    steps_d = din("steps", (128, 4 * 512))      # [r, p*512+q] = [p*128+r > q]
    negeye_d = din("negeye", (128, 128))        # -1e30 * I
    out_d = nc.dram_tensor("out_partial", [S, HID], bf,
                           kind="ExternalOutput").ap()

    cc1_in = nc.dram_tensor("cc1_in", [KVLR + 128, SL], bf).ap()
    cc1_out = nc.dram_tensor("cc1_out", [NC_N * (KVLR + 128), SL], bf,
                             addr_space="Shared").ap()
    cc2_in = nc.dram_tensor("cc2_in", [QLR, SL], bf).ap()
    cc2_out = nc.dram_tensor("cc2_out", [NC_N * QLR, SL], bf,
                             addr_space="Shared").ap()

    with tile.TileContext(nc) as tc, ExitStack() as ctx:
        def pool(name, bufs):
            return ctx.enter_context(tc.tile_pool(name=name, bufs=bufs))

        p_hs = pool("hs", 3)
        p_w = pool("wstr", 3)
        p_raw = pool("raw", 12)
        p_sq = pool("sqt", 2)
        p_scn = pool("scn", 2)
        p_sml = pool("sml", 4)
        p_one = pool("ones", 2)
        p_cs = pool("cs", 2)
        p_csl = pool("csl", 2)
        p_bc = pool("bc", 2)
        p_kc = pool("kc", 1)
        p_vc = pool("vc", 1)
        p_wqb = pool("wqb", 2)
        p_wo = pool("wo", 5)
        p_msk = pool("msk", 1)
        p_kpeg = pool("kpeg", 4)
        p_K = pool("Kt", 4)
        p_V = pool("Vt", 16)
        p_qn = pool("Qn", 4)
        p_rope = pool("rope", 2)
        p_f32 = pool("fr32", 2)
        p_ckvg = pool("ckvg", 16)
        p_wk = pool("wk", 14)       # shared ring: qlg -> P tiles
        p_pacc = pool("pacc", 2)
        p_oT = pool("oT", 4)
        p_out = pool("outst", 3)

        pp_o = ctx.enter_context(
            tc.tile_pool(name="ppo", bufs=4, space="PSUM"))
        pp_s = ctx.enter_context(
            tc.tile_pool(name="pps", bufs=4, space="PSUM"))
        pp_sm = pp_s

        ones_col = p_one.tile([128, 1], bf)       # lhsT for column sums
        nc.vector.memset(ones_col[:], 1.0)
        eps_t = p_one.tile([1, 1], f32, tag="eps", name="eps")
        nc.vector.memset(eps_t[:], EPS)

        def rope_apply(dst, src_ps, cos_t, sin_t, c0, width, u1, u2):
            """dst(bf16) = src*cos + rot(src)*sin_signed, via partition-offset
            muls. src_ps is a [128, width] f32 PSUM pack of 64-dim halves."""
            nc.vector.tensor_mul(u1[:, :width], src_ps[:],
                                 cos_t[:, c0:c0 + width])
            for blk in (0, 64):
                nc.vector.tensor_mul(
                    u2[blk:blk + 32, :width],
                    src_ps[blk + 32:blk + 64, :],
                    sin_t[blk:blk + 32, c0:c0 + width])
                nc.vector.tensor_mul(
                    u2[blk + 32:blk + 64, :width],
                    src_ps[blk:blk + 32, :],
                    sin_t[blk + 32:blk + 64, c0:c0 + width])
            nc.vector.tensor_add(dst, u1[:, :width], u2[:, :width])

        # ---------------- FRONT 1: ckv + k_pe pass (then AG1) --------------
        ckv_ps = [pp_o.tile([128, SL], f32, tag="po", name="po")
                  for _ in range(KC)]
        pe_ps = pp_s.tile([128, SL], f32, tag="ps", name="ps")
        for k2 in range(K2):
            hs2 = p_hs.tile([128, 2 * SL], bf, tag="hs", name="hs")
            nc.sync.dma_start(hs2[:], hsT2_d[:, k2 * 512:(k2 + 1) * 512])
            wv = p_w.tile([128, 1280], bf, tag="wkv", name="wkv", bufs=3)
            nc.scalar.dma_start(wv[:], wkvpe2_d[:, k2 * 1280:(k2 + 1) * 1280])
            for kk in range(2):
                k = 2 * k2 + kk
                hh = hs2[:, kk * SL:(kk + 1) * SL]
                wk0 = kk * 640
                for c in range(KC):
                    nc.tensor.matmul(ckv_ps[c][:],
                                     wv[:, wk0 + c * 128:wk0 + (c + 1) * 128],
                                     hh, start=(k == 0), stop=(k == KH - 1))
                nc.tensor.matmul(pe_ps[:], wv[:, wk0 + KVLR:wk0 + KVLR + 128],
                                 hh,
                                 start=(k == 0), stop=(k == KH - 1))
        ssq_kv = pp_sm.tile([1, SL], f32, tag="ps", name="ps")
        raw_kv = []
        for c in range(KC):
            r = p_raw.tile([128, SL], bf, tag="raw", name="raw")
            nc.scalar.activation(r[:], ckv_ps[c][:], AF.Copy)
            raw_kv.append(r)
            sq = p_sq.tile([128, SL], bf, tag="sq", name="sq")
            nc.scalar.activation(sq[:], ckv_ps[c][:], AF.Square)
            nc.tensor.matmul(ssq_kv[:], ones_col[:], sq[:],
                             start=(c == 0), stop=(c == KC - 1))
        t_kv = p_sml.tile([1, SL], f32, tag="sml", name="sml")
        nc.scalar.activation(t_kv[:], ssq_kv[:], AF.Sqrt,
                             bias=eps_t[:], scale=1.0 / KVLR)
        s_kv = p_sml.tile([1, SL], f32, tag="sml", name="sml")
        nc.vector.reciprocal(s_kv[:], t_kv[:])
        bkv_sb = p_bc.tile([128, 512], f32, tag="bc", name="bc")
        nc.gpsimd.partition_broadcast(bkv_sb[:, :SL], s_kv[:])
        for c in range(KC):
            cn = p_scn.tile([128, SL], bf, tag="scn", name="scn")
            nc.vector.tensor_mul(cn[:], raw_kv[c][:], bkv_sb[:, :SL])
            nc.scalar.dma_start(cc1_in[c * 128:(c + 1) * 128, :], cn[:])
        # rope k_pe (partition-offset rotation, sign folded into sin table)
        cosl_t = p_csl.tile([128, SL], bf, tag="csl", name="csl")
        sinl_t = p_csl.tile([128, SL], bf, tag="csl", name="csl")
        nc.sync.dma_start(cosl_t[:], cosl_d[:, :])
        nc.sync.dma_start(sinl_t[:], sinlsg_d[:, :])
        u1f = p_f32.tile([128, 512], f32, tag="f32", name="f32")
        u2f = p_f32.tile([128, 512], f32, tag="f32", name="f32")
        kpe_n = p_scn.tile([128, SL], bf, tag="scn", name="scn")
        rope_apply(kpe_n[:], pe_ps, cosl_t, sinl_t, 0, SL, u1f, u2f)
        nc.scalar.dma_start(cc1_in[KVLR:KVLR + 128, :], kpe_n[:])

        nc.gpsimd.collective_compute(
            "AllGather", mybir.AluOpType.bypass,
            ins=[cc1_in[:]], outs=[cc1_out[:]],
            replica_groups=[list(range(NC_N))],
        )

        # ---------------- FRONT 2: q_lora pass (then AG2) ------------------
        # resident back-end weights stream on the scalar/gpsimd rings
        kct_t = p_kc.tile([128, HPC * KC * 128], bf, tag="kc", name="kc")
        nc.scalar.dma_start(kct_t[:], kct2_d[:, :])
        vcp_t = p_vc.tile([128, KC * HPC * DV], bf, tag="vc", name="vc")
        nc.gpsimd.dma_start(vcp_t[:], vcp2_d[:, :])
        wqb_t = []
        for h in range(2):
            t = p_wqb.tile([128, 6 * 768], bf, tag="wqb", name="wqb")
            eng = nc.scalar if h == 0 else nc.gpsimd
            eng.dma_start(t[:], wqb2_d[:, h * 6 * 768:(h + 1) * 6 * 768])
            wqb_t.append(t)
        cosf_t = p_cs.tile([128, S], bf, tag="cs", name="cs")
        sinf_t = p_cs.tile([128, S], bf, tag="cs", name="cs")
        nc.scalar.dma_start(cosf_t[:], cosf_d[:, :])
        nc.gpsimd.dma_start(sinf_t[:], sinfsg_d[:, :])
        steps_t = p_msk.tile([128, 4 * 512], bf, tag="msk", name="msk")
        nc.scalar.dma_start(steps_t[:], steps_d[:, :])
        negeye_t = p_msk.tile([128, 128], bf, tag="ne", name="ne")
        nc.gpsimd.dma_start(negeye_t[:], negeye_d[:, :])

        def wqb_ap(k, c0, c1):
            return wqb_t[k // 6][:, (k % 6) * 768 + c0:(k % 6) * 768 + c1]

        ssq_g = []
        raw_q = []
        for g in range(2):
            ql_ps = ([pp_o.tile([128, SL], f32, tag="po", name="po")
                      for _ in range(4)] +
                     [pp_s.tile([128, SL], f32, tag="ps", name="ps")
                      for _ in range(2)])
            for k2 in range(K2):
                hs2 = p_hs.tile([128, 2 * SL], bf, tag="hs", name="hs")
                heng = nc.sync if k2 % 2 == 0 else nc.scalar
                heng.dma_start(hs2[:], hsT2_d[:, k2 * 512:(k2 + 1) * 512])
                wq = []
                for kk in range(2):
                    k = 2 * k2 + kk
                    w = p_w.tile([128, QLR // 2], bf, tag="wqa", name="wqa",
                                 bufs=6)
                    eng = nc.sync if kk == 0 else nc.scalar
                    eng.dma_start(
                        w[:], wqa_d[k * 128:(k + 1) * 128,
                                    g * (QLR // 2):(g + 1) * (QLR // 2)])
                    wq.append(w)
                for kk in range(2):
                    k = 2 * k2 + kk
                    hh = hs2[:, kk * SL:(kk + 1) * SL]
                    for mi in range(6):
                        nc.tensor.matmul(ql_ps[mi][:],
                                         wq[kk][:, mi * 128:(mi + 1) * 128],
                                         hh, start=(k == 0),
                                         stop=(k == KH - 1))
            ssq_gt = pp_sm.tile([1, SL], f32, tag="ps", name="ps")
            ssq_g.append(ssq_gt)
            for mi in range(6):
                r = p_raw.tile([128, SL], bf, tag="raw", name="raw")
                nc.scalar.activation(r[:], ql_ps[mi][:], AF.Copy)
                raw_q.append(r)
                sq = p_sq.tile([128, SL], bf, tag="sq", name="sq")
                nc.scalar.activation(sq[:], ql_ps[mi][:], AF.Square)
                nc.tensor.matmul(ssq_gt[:], ones_col[:], sq[:],
                                 start=(mi == 0), stop=(mi == 5))
        ssg0 = p_sml.tile([1, SL], f32, tag="sml", name="sml")
        nc.scalar.activation(ssg0[:], ssq_g[0][:], AF.Copy)
        ssum_q = p_sml.tile([1, SL], f32, tag="sml", name="sml")
        nc.vector.tensor_add(ssum_q[:], ssg0[:], ssq_g[1][:])
        t_q = p_sml.tile([1, SL], f32, tag="sml", name="sml")
        nc.scalar.activation(t_q[:], ssum_q[:], AF.Sqrt,
                             bias=eps_t[:], scale=1.0 / QLR)
        s_q = p_sml.tile([1, SL], f32, tag="sml", name="sml")
        nc.vector.reciprocal(s_q[:], t_q[:])
        bq_sb = p_bc.tile([128, 512], f32, tag="bc", name="bc")
        nc.gpsimd.partition_broadcast(bq_sb[:, :SL], s_q[:])
        for m in range(KQ):
            qn = p_scn.tile([128, SL], bf, tag="scn", name="scn")
            nc.vector.tensor_mul(qn[:], raw_q[m][:], bq_sb[:, :SL])
            nc.scalar.dma_start(cc2_in[m * 128:(m + 1) * 128, :], qn[:])

        nc.gpsimd.collective_compute(
            "AllGather", mybir.AluOpType.bypass,
            ins=[cc2_in[:]], outs=[cc2_out[:]],
            replica_groups=[list(range(NC_N))],
        )

        # ---------------- BACK: K/V expansion (needs AG1 only) -------------
        RPC = 512 // SL     # AG rank-blocks per 512-wide seq chunk
        g_engs = [nc.scalar, nc.gpsimd, nc.sync]
        g_rr = [0]

        def rr_eng():
            e = g_engs[g_rr[0] % 3]
            g_rr[0] += 1
            return e

        ckvg = {}
        for c in range(KC):
            for sc in range(SC):
                t = p_ckvg.tile([128, 512], bf, tag="ckvg", name="ckvg")
                for half in range(RPC):
                    r = RPC * sc + half
                    rr_eng().dma_start(
                        t[:, half * SL:(half + 1) * SL],
                        cc1_out[r * (KVLR + 128) + c * 128:
                                r * (KVLR + 128) + (c + 1) * 128, :])
                ckvg[(c, sc)] = t
        kpeg = {}
        for sc in range(SC):
            t = p_kpeg.tile([128, 512], bf, tag="kpeg", name="kpeg")
            for half in range(RPC):
                r = RPC * sc + half
                rr_eng().dma_start(
                    t[:, half * SL:(half + 1) * SL],
                    cc1_out[r * (KVLR + 128) + KVLR:
                            r * (KVLR + 128) + KVLR + 128, :])
            kpeg[sc] = t
        # w_o resident tiles: streamed after the gathers
        wo_t = []
        for h in range(5):
            t = p_wo.tile([128, 8 * 512], bf, tag="wo", name="wo")
            eng = g_engs[h % 2]
            eng.dma_start(t[:], wo2_d[:, h * 8 * 512:(h + 1) * 8 * 512])
            wo_t.append(t)

        def wo_ap(i, n):
            b = i * NW + n
            return wo_t[b // 8][:, (b % 8) * 512:(b % 8 + 1) * 512]

        # K^T per head: [DN, S]
        K_t = []
        for i in range(HPC):
            kt = p_K.tile([128, S], bf, tag="K", name="K")
            K_t.append(kt)
            for sc in range(SC):
                ps = pp_s.tile([128, 512], f32, tag="ps", name="ps")
                for c in range(KC):
                    nc.tensor.matmul(
                        ps[:],
                        kct_t[:, (i * KC + c) * 128:(i * KC + c + 1) * 128],
                        ckvg[(c, sc)][:],
                        start=(c == 0), stop=(c == KC - 1))
                nc.scalar.activation(kt[:, sc * 512:(sc + 1) * 512], ps[:],
                                     AF.Copy)

        # V natural: per seq-block [128, 4*DV]
        V_t = []
        for sb in range(SB):
            ps = pp_s.tile([128, 512], f32, tag="ps", name="ps")
            for c in range(KC):
                nc.tensor.matmul(
                    ps[:],
                    ckvg[(c, sb // 4)][:, (sb % 4) * 128:(sb % 4 + 1) * 128],
                    vcp_t[:, c * 512:(c + 1) * 512],
                    start=(c == 0), stop=(c == KC - 1))
            vt = p_V.tile([128, HPC * DV], bf, tag="V", name="V")
            nc.scalar.activation(vt[:], ps[:], AF.Copy)
            V_t.append(vt)

        # ---------------- BACK: per-chunk Q proj + attention + w_o ---------
        qg_engs = [nc.gpsimd, nc.scalar]

        def gather_qlg(sc):
            qlg = []
            for k in range(KQ):
                t = p_wk.tile([128, 512], bf, tag="wk", name="wk")
                for half in range(RPC):
                    r = RPC * sc + half
                    qg_engs[(2 * k + half) % 2].dma_start(
                        t[:, half * SL:(half + 1) * SL],
                        cc2_out[r * QLR + k * 128:r * QLR + (k + 1) * 128, :])
                qlg.append(t)
            return qlg

        qlg = gather_qlg(0)
        for sc in range(SC):
            # --- Q^T nope per head + pe packs (roped) ---
            qn_t = []
            for i in range(HPC):
                ps = pp_o.tile([128, 512], f32, tag="po", name="po")
                for k in range(KQ):
                    nc.tensor.matmul(ps[:], wqb_ap(k, i * 128, (i + 1) * 128),
                                     qlg[k][:], start=(k == 0),
                                     stop=(k == KQ - 1))
                qt = p_qn.tile([128, 512], bf, tag="Qn", name="Qn")
                nc.scalar.activation(qt[:], ps[:], AF.Copy)
                qn_t.append(qt)
            roped = []
            for pkt in range(2):
                ps_pe = pp_s.tile([128, 512], f32, tag="ps", name="ps")
                for k in range(KQ):
                    nc.tensor.matmul(
                        ps_pe[:],
                        wqb_ap(k, 512 + pkt * 128, 512 + (pkt + 1) * 128),
                        qlg[k][:], start=(k == 0), stop=(k == KQ - 1))
                u1 = p_f32.tile([128, 512], f32, tag="f32", name="f32")
                u2 = p_f32.tile([128, 512], f32, tag="f32", name="f32")
                rp = p_rope.tile([128, 512], bf, tag="rope", name="rope")
                rope_apply(rp[:], ps_pe, cosf_t, sinf_t, sc * 512, 512, u1, u2)
                roped.append(rp)

            # --- attention, software-pipelined over j (lookahead 2) ---
            o_ps_l, dinv_l = [], []
            for i in range(HPC):
                pkt, hp = i // 2, i % 2
                o_ps = pp_o.tile([128, 512], f32, tag="po", name="po")
                o_ps_l.append(o_ps)
                pacc = p_pacc.tile([128, 512], bf, tag="pacc", name="pacc")
                nj = 4 * sc + 4
                pts = []

                def consume(j):
                    nc.tensor.matmul(o_ps[:],
                                     V_t[j][:, i * DV:(i + 1) * DV],
                                     pts[j][:], start=(j == 0),
                                     stop=(j == nj - 1))
                    if j == 0:
                        nc.vector.tensor_copy(pacc[:], pts[j][:])
                    else:
                        nc.vector.tensor_add(pacc[:], pacc[:], pts[j][:])

                for j in range(nj):
                    s_ps = pp_s.tile([128, 512], f32, tag="ps", name="ps")
                    nc.tensor.matmul(s_ps[:],
                                     K_t[i][:, j * 128:(j + 1) * 128],
                                     qn_t[i][:], start=True, stop=False)
                    if j >= 4 * sc:
                        p = j - 4 * sc
                        nc.tensor.matmul(
                            s_ps[:], negeye_t[:],
                            steps_t[:, p * 512:(p + 1) * 512],
                            start=False, stop=False)
                    nc.tensor.matmul(
                        s_ps[:],
                        kpeg[j // 4][hp * 64:(hp + 1) * 64,
                                     (j % 4) * 128:(j % 4 + 1) * 128],
                        roped[pkt][hp * 64:(hp + 1) * 64, :],
                        start=False, stop=True)
                    pt = p_wk.tile([128, 512], bf, tag="wk", name="wk")
                    nc.scalar.activation(pt[:], s_ps[:], AF.Exp)
                    pts.append(pt)
                    if j >= 3:
                        consume(j - 3)
                for j in range(max(0, nj - 3), nj):
                    consume(j)
                d_ps = pp_sm.tile([1, 512], f32, tag="ps", name="ps")
                nc.tensor.matmul(d_ps[:], ones_col[:], pacc[:],
                                 start=True, stop=True)
                dinv = p_sml.tile([1, 512], f32, tag="sml", name="sml")
                nc.vector.reciprocal_approx_fast(dinv[:], d_ps[:])
                dinv_l.append(dinv)

            oT = []
            for i in range(HPC):
                bc_sb = p_bc.tile([128, 512], f32, tag="bc", name="bc")
                nc.gpsimd.partition_broadcast(bc_sb[:], dinv_l[i][:])
                ot = p_oT.tile([128, 512], bf, tag="oT", name="oT")
                nc.vector.tensor_mul(ot[:], o_ps_l[i][:], bc_sb[:])
                oT.append(ot)

            # prefetch next chunk's gathered q_lora during w_o
            if sc + 1 < SC:
                qlg = gather_qlg(sc + 1)

            # --- w_o partial for this seq chunk ---
            # sbl outer / paired n inner: two [128,512] psum groups stage
            # into one [128,1024] tile and ship with a single DMA
            for sbl in range(4):
                sb = sc * 4 + sbl
                for n2 in range(NW // 2):
                    ob = p_out.tile([128, 1024], bf, tag="outst",
                                    name="outst")
                    for half in range(2):
                        n = 2 * n2 + half
                        ps = pp_s.tile([128, 512], f32, tag="ps", name="ps")
                        for i in range(HPC):
                            nc.tensor.matmul(
                                ps[:], oT[i][:, sbl * 128:(sbl + 1) * 128],
                                wo_ap(i, n), start=(i == 0),
                                stop=(i == HPC - 1))
                        ceng = nc.scalar if half == 0 else nc.vector
                        if half == 0:
                            ceng.activation(ob[:, :512], ps[:], AF.Copy)
                        else:
                            ceng.tensor_copy(ob[:, 512:], ps[:])
                    nc.sync.dma_start(
                        out_d[sb * 128:(sb + 1) * 128,
                              n2 * 1024:(n2 + 1) * 1024], ob[:])

    nc.compile()
    return nc


def _prep_inputs(inputs):
    """Host-side sharding + weight folding. Returns in_maps (list of 8 dicts)."""
    BF = _bf16()

    hs = np.asarray(inputs['hidden_states'], np.float32)
    pos = np.asarray(inputs['positions'])
    w_qa = np.asarray(inputs['w_qa'], np.float32)
    q_a_ln_w = np.asarray(inputs['q_a_ln_w'], np.float32)
    w_qb = np.asarray(inputs['w_qb'], np.float32)
    w_kva = np.asarray(inputs['w_kva'], np.float32)
    kv_a_ln_w = np.asarray(inputs['kv_a_ln_w'], np.float32)
    kc = np.asarray(inputs['kc'], np.float32)
    vc = np.asarray(inputs['vc'], np.float32)
    w_o = np.asarray(inputs['w_o'], np.float32)

    perm = np.concatenate([np.arange(0, DR, 2), np.arange(1, DR, 2)])
    inv_freq = 1.0 / (ROPE_BASE ** (np.arange(0, DR, 2, dtype=np.float64) / DR))
    freqs = pos.astype(np.float64)[None, :] * inv_freq[:, None]     # [32, S]
    cosT = np.cos(freqs).astype(np.float32)
    sinT = np.sin(freqs).astype(np.float32)
    cos128 = np.tile(cosT, (4, 1)).astype(BF)                        # [128, S]
    sin128 = np.tile(sinT, (4, 1)).astype(np.float32)
    sgn = np.where((np.arange(128) % 64) < 32, -1.0, 1.0)[:, None]
    sinsg128 = (sin128 * sgn).astype(BF)

    scale = DQ ** -0.5
    w_qb_eff = ((w_qb * q_a_ln_w[:, None]) * scale).reshape(QLR, H, DQ)

    w_pe = w_kva[:, KVLR:][:, perm]
    wkvpe = np.concatenate([w_kva[:, :KVLR], w_pe, w_pe], 1)   # [HID, 640]
    K2h = (HID // 128) // 2
    wkvpe2 = wkvpe.reshape(K2h, 2, 128, 640).transpose(2, 0, 1, 3) \
        .reshape(128, K2h * 1280).astype(BF)

    kc_f = kc * kv_a_ln_w[None, None, :]
    vc_f = vc * kv_a_ln_w[None, :, None]

    # step tables: steps[r, p*512+q] = 1 if p*128+r > q else 0
    steps = np.zeros((128, 4 * 512), np.float32)
    rr = np.arange(128)[:, None]
    qq = np.arange(512)[None, :]
    for p in range(4):
        steps[:, p * 512:(p + 1) * 512] = (p * 128 + rr > qq)
    steps_b = steps.astype(BF)
    negeye = (-1e30 * np.eye(128, dtype=np.float32)).astype(BF)

    wqa_b = w_qa.astype(BF)

    K2 = (HID // 128) // 2
    NW = HID // 512

    in_maps = []
    for core in range(NC_N):
        rows = slice(core * SL, (core + 1) * SL)
        h0 = core * HPC

        hsT = np.ascontiguousarray(hs[rows].T)                   # [HID, SL]
        hsT2 = hsT.reshape(K2, 2, 128, SL).transpose(2, 0, 1, 3) \
            .reshape(128, K2 * 2 * SL)

        wqb_all = np.empty((QLR, 768), np.float32)
        for i in range(HPC):
            wqb_all[:, i * 128:(i + 1) * 128] = w_qb_eff[:, h0 + i, :DN]
        for pkt in range(2):
            a, b = h0 + 2 * pkt, h0 + 2 * pkt + 1
            pe_a = w_qb_eff[:, a, DN:][:, perm]
            pe_b = w_qb_eff[:, b, DN:][:, perm]
            wqb_all[:, 512 + pkt * 128:512 + pkt * 128 + 64] = pe_a
            wqb_all[:, 512 + pkt * 128 + 64:512 + (pkt + 1) * 128] = pe_b
        wqb2 = wqb_all.reshape(12, 128, 768).transpose(1, 0, 2) \
            .reshape(128, 12 * 768)

        kct = np.stack([kc_f[h0 + i].T[c * 128:(c + 1) * 128]
                        for i in range(HPC) for c in range(KVLR // 128)])
        kct2 = kct.transpose(1, 0, 2).reshape(128, -1)           # [128, 2048]

        vcp = np.concatenate([vc_f[h0 + i] for i in range(HPC)], 1)
        vcp2 = vcp.reshape(KVLR // 128, 128, HPC * DV) \
            .transpose(1, 0, 2).reshape(128, -1)                 # [128, 2048]

        wo_sh = w_o[h0 * DV:(h0 + HPC) * DV, :]                  # [512, HID]
        wo2 = wo_sh.reshape(HPC, 128, NW, 512).transpose(0, 2, 1, 3) \
            .reshape(HPC * NW, 128, 512).transpose(1, 0, 2) \
            .reshape(128, -1)                                    # [128, 20480]

        in_maps.append({
            "hsT2": hsT2.astype(BF),
            "wqa": wqa_b,
            "wkvpe2": wkvpe2,
            "cosl": np.ascontiguousarray(cos128[:, rows]),
            "sinlsg": np.ascontiguousarray(sinsg128[:, rows]),
            "cosf": cos128,
            "sinfsg": sinsg128,
            "wqb2": wqb2.astype(BF),
            "kct2": kct2.astype(BF),
            "vcp2": vcp2.astype(BF),
            "wo2": wo2.astype(BF),
            "steps": steps_b,
            "negeye": negeye,
        })
    return in_maps


def _get_program():
    if "nc" not in _CACHE:
        _CACHE["nc"] = _build_program()
    return _CACHE["nc"]


def run(inputs, trace=False, trace_kwargs=None):
    """Build (cached), run on 8 cores, return (output, BassKernelResults)."""
    from concourse.bass_utils import run_bass_kernel_spmd

    nc = _get_program()
    in_maps = _prep_inputs(inputs)
    res = run_bass_kernel_spmd(nc, in_maps, list(range(NC_N)),
                               trace=trace, **(trace_kwargs or {}))
    out = np.zeros((S, HID), np.float32)
    for r in res.results:
        out += np.asarray(r["out_partial"], dtype=np.float32)
    return out, res


def kernel(**inputs) -> np.ndarray:
    out, _ = run(inputs, trace=False)
    return out


# revision 30
# speedup vs baseline: 1.1036x; 1.0057x over previous
"""Trainium2 Bass kernel for a DeepseekV2 decoder-layer attention block
(MLA prefill, fp32 reference) distributed across 8 NeuronCores.

Strategy (single NEFF, SPMD on 8 cores):
  - Sequence-shard the shared projections: each core computes ckv / k_pe
    (RMS-normed / roped) then q_lora for its 256 rows of the sequence, in
    transposed layout; two on-device AllGathers replicate them. ckv goes
    first so its AllGather flies under the q_lora GEMM; the q_lora
    AllGather flies under the K/V expansion.
  - Head-shard the rest (4 heads per core): q_b projection + RoPE, kc/vc
    expansion, causal attention (scores computed transposed so the attn@v
    matmul needs no transposes), and a row-shard of w_o.
  - Each core emits a partial [S, HID] bf16 output; the host sums the 8
    partials (the output all-reduce) to produce the full result.

Perf structure (v3):
  - All weight/activation streams are host-packed into [128, N] layouts so
    every DMA is one large transfer, spread round-robin across the
    sync/scalar/gpsimd/vector DGE rings (the per-DMA ~600ns issue cost made
    the v2 front end ring-bound).
  - Attention is software-pipelined (lookahead 2); the causal mask is
    applied as a third matmul into the score PSUM group (-1e30 * step),
    so the exp -> attn@v chain has no vector-engine hop.
  - Softmax denominators: bf16 P-tile running sum on the vector engine,
    one ones-matmul per (head, chunk), reciprocal_approx_fast.
  - RoPE rotation via partition-offset vector ops with sign-folded sin.
  - w_o resident; output staged bf16 through the scalar engine.
"""

import numpy as np

S, HID, H = 2048, 5120, 32
QLR, KVLR = 1536, 512
DN, DR, DV = 128, 64, 128
DQ = DN + DR
NC_N = 8
HPC = H // NC_N          # heads per core
SL = S // NC_N           # sequence rows per core (front end)
ROPE_BASE, EPS = 10000.0, 1e-6

_CACHE = {}


def _bf16():
    import ml_dtypes
    return np.dtype(ml_dtypes.bfloat16)


def _build_program():
    import concourse.bass as bass
    import concourse.tile as tile
    from concourse import bacc, mybir
    from contextlib import ExitStack

    f32 = mybir.dt.float32
    bf = mybir.dt.bfloat16
    AF = mybir.ActivationFunctionType

    nc = bacc.Bacc("TRN2", target_bir_lowering=False, debug=False,
                   num_devices=NC_N)

    def din(name, shape, dt=bf):
        return nc.dram_tensor(name, list(shape), dt, kind="ExternalInput").ap()

    KH = HID // 128       # 40 k-chunks of the model dim
    K2 = KH // 2          # 20 double-chunks
    KQ = QLR // 128       # 12 chunks of the q-lora dim
    KC = KVLR // 128      # 4 chunks of the kv-lora dim
    SC = S // 512         # 4 sequence chunks of 512
    SB = S // 128         # 16 sequence blocks of 128
    NW = HID // 512       # 10 w_o column chunks

    hsT2_d = din("hsT2", (128, K2 * 2 * SL))    # [p, k2*512+half*256+c]
    wqa_d = din("wqa", (HID, QLR))
    wkvpe2_d = din("wkvpe2", (128, (HID // 256) * 1280))  # packed pairs
    cosl_d = din("cosl", (128, SL))
    sinlsg_d = din("sinlsg", (128, SL))         # sign-folded sin
    cosf_d = din("cosf", (128, S))
    sinfsg_d = din("sinfsg", (128, S))
    wqb2_d = din("wqb2", (128, KQ * 768))       # [p, k*768 + col]
    kct2_d = din("kct2", (128, HPC * KC * 128))  # [p, (i*4+c)*128 + d]
    vcp2_d = din("vcp2", (128, KC * HPC * DV))  # [p, c*512 + col]
    wo2_d = din("wo2", (128, HPC * NW * 512))   # [p, (i*NW+n)*512 + col]
    steps_d = din("steps", (128, 4 * 512))      # [r, p*512+q] = [p*128+r > q]
    negeye_d = din("negeye", (128, 128))        # -1e30 * I
    out_d = nc.dram_tensor("out_partial", [S, HID], bf,
                           kind="ExternalOutput").ap()

    cc1_in = nc.dram_tensor("cc1_in", [KVLR + 128, SL], bf).ap()
    cc1_out = nc.dram_tensor("cc1_out", [NC_N * (KVLR + 128), SL], bf,
                             addr_space="Shared").ap()
    cc2_in = nc.dram_tensor("cc2_in", [QLR, SL], bf).ap()
    cc2_out = nc.dram_tensor("cc2_out", [NC_N * QLR, SL], bf,
                             addr_space="Shared").ap()

    with tile.TileContext(nc) as tc, ExitStack() as ctx:
        def pool(name, bufs):
            return ctx.enter_context(tc.tile_pool(name=name, bufs=bufs))

        p_hs = pool("hs", 4)
        p_w = pool("wstr", 3)
        p_raw = pool("raw", 12)
        p_sq = pool("sqt", 1)
        p_scn = pool("scn", 2)
        p_sml = pool("sml", 4)
        p_one = pool("ones", 2)
        p_cs = pool("cs", 2)
        p_csl = pool("csl", 2)
        p_bc = pool("bc", 2)
        p_kc = pool("kc", 1)
        p_vc = pool("vc", 1)
        p_wqb = pool("wqb", 2)
        p_wo = pool("wo", 5)
        p_msk = pool("msk", 1)
        p_kpeg = pool("kpeg", 4)
        p_K = pool("Kt", 4)
        p_V = pool("Vt", 16)
        p_qn = pool("Qn", 4)
        p_rope = pool("rope", 2)
        p_f32 = pool("fr32", 2)
        p_ckvg = pool("ckvg", 16)
        p_wk = pool("wk", 14)       # shared ring: qlg -> P tiles
        p_pacc = pool("pacc", 2)
        p_oT = pool("oT", 4)
        p_out = pool("outst", 2)

        pp_o = ctx.enter_context(
            tc.tile_pool(name="ppo", bufs=4, space="PSUM"))
        pp_s = ctx.enter_context(
            tc.tile_pool(name="pps", bufs=4, space="PSUM"))
        pp_sm = pp_s

        ones_col = p_one.tile([128, 1], bf)       # lhsT for column sums
        nc.vector.memset(ones_col[:], 1.0)
        eps_t = p_one.tile([1, 1], f32, tag="eps", name="eps")
        nc.vector.memset(eps_t[:], EPS)

        def rope_apply(dst, src_ps, cos_t, sin_t, c0, width, u1, u2):
            """dst(bf16) = src*cos + rot(src)*sin_signed, via partition-offset
            muls. src_ps is a [128, width] f32 PSUM pack of 64-dim halves."""
            nc.vector.tensor_mul(u1[:, :width], src_ps[:],
                                 cos_t[:, c0:c0 + width])
            for blk in (0, 64):
                nc.vector.tensor_mul(
                    u2[blk:blk + 32, :width],
                    src_ps[blk + 32:blk + 64, :],
                    sin_t[blk:blk + 32, c0:c0 + width])
                nc.vector.tensor_mul(
                    u2[blk + 32:blk + 64, :width],
                    src_ps[blk:blk + 32, :],
                    sin_t[blk + 32:blk + 64, c0:c0 + width])
            nc.vector.tensor_add(dst, u1[:, :width], u2[:, :width])

        # ---------------- FRONT 1: ckv + k_pe pass (then AG1) --------------
        ckv_ps = [pp_o.tile([128, SL], f32, tag="po", name="po")
                  for _ in range(KC)]
        pe_ps = pp_s.tile([128, SL], f32, tag="ps", name="ps")
        for k2 in range(K2):
            hs2 = p_hs.tile([128, 2 * SL], bf, tag="hs", name="hs")
            nc.sync.dma_start(hs2[:], hsT2_d[:, k2 * 512:(k2 + 1) * 512])
            wv = p_w.tile([128, 1280], bf, tag="wkv", name="wkv", bufs=3)
            nc.scalar.dma_start(wv[:], wkvpe2_d[:, k2 * 1280:(k2 + 1) * 1280])
            for kk in range(2):
                k = 2 * k2 + kk
                hh = hs2[:, kk * SL:(kk + 1) * SL]
                wk0 = kk * 640
                for c in range(KC):
                    nc.tensor.matmul(ckv_ps[c][:],
                                     wv[:, wk0 + c * 128:wk0 + (c + 1) * 128],
                                     hh, start=(k == 0), stop=(k == KH - 1))
                nc.tensor.matmul(pe_ps[:], wv[:, wk0 + KVLR:wk0 + KVLR + 128],
                                 hh,
                                 start=(k == 0), stop=(k == KH - 1))
        ssq_kv = pp_sm.tile([1, SL], f32, tag="ps", name="ps")
        raw_kv = []
        for c in range(KC):
            r = p_raw.tile([128, SL], bf, tag="raw", name="raw")
            nc.scalar.activation(r[:], ckv_ps[c][:], AF.Copy)
            raw_kv.append(r)
            sq = p_sq.tile([128, SL], bf, tag="sq", name="sq")
            nc.scalar.activation(sq[:], ckv_ps[c][:], AF.Square)
            nc.tensor.matmul(ssq_kv[:], ones_col[:], sq[:],
                             start=(c == 0), stop=(c == KC - 1))
        t_kv = p_sml.tile([1, SL], f32, tag="sml", name="sml")
        nc.scalar.activation(t_kv[:], ssq_kv[:], AF.Sqrt,
                             bias=eps_t[:], scale=1.0 / KVLR)
        s_kv = p_sml.tile([1, SL], f32, tag="sml", name="sml")
        nc.vector.reciprocal(s_kv[:], t_kv[:])
        bkv_sb = p_bc.tile([128, 512], f32, tag="bc", name="bc")
        nc.gpsimd.partition_broadcast(bkv_sb[:, :SL], s_kv[:])
        for c in range(KC):
            cn = p_scn.tile([128, SL], bf, tag="scn", name="scn")
            nc.vector.tensor_mul(cn[:], raw_kv[c][:], bkv_sb[:, :SL])
            nc.scalar.dma_start(cc1_in[c * 128:(c + 1) * 128, :], cn[:])
        # rope k_pe (partition-offset rotation, sign folded into sin table)
        cosl_t = p_csl.tile([128, SL], bf, tag="csl", name="csl")
        sinl_t = p_csl.tile([128, SL], bf, tag="csl", name="csl")
        nc.sync.dma_start(cosl_t[:], cosl_d[:, :])
        nc.sync.dma_start(sinl_t[:], sinlsg_d[:, :])
        u1f = p_f32.tile([128, 512], f32, tag="f32", name="f32")
        u2f = p_f32.tile([128, 512], f32, tag="f32", name="f32")
        kpe_n = p_scn.tile([128, SL], bf, tag="scn", name="scn")
        rope_apply(kpe_n[:], pe_ps, cosl_t, sinl_t, 0, SL, u1f, u2f)
        nc.scalar.dma_start(cc1_in[KVLR:KVLR + 128, :], kpe_n[:])

        nc.gpsimd.collective_compute(
            "AllGather", mybir.AluOpType.bypass,
            ins=[cc1_in[:]], outs=[cc1_out[:]],
            replica_groups=[list(range(NC_N))],
        )

        # ---------------- FRONT 2: q_lora pass (then AG2) ------------------
        # resident back-end weights stream on the scalar/gpsimd rings
        kct_t = p_kc.tile([128, HPC * KC * 128], bf, tag="kc", name="kc")
        nc.scalar.dma_start(kct_t[:], kct2_d[:, :])
        vcp_t = p_vc.tile([128, KC * HPC * DV], bf, tag="vc", name="vc")
        nc.gpsimd.dma_start(vcp_t[:], vcp2_d[:, :])
        wqb_t = []
        for h in range(2):
            t = p_wqb.tile([128, 6 * 768], bf, tag="wqb", name="wqb")
            eng = nc.scalar if h == 0 else nc.gpsimd
            eng.dma_start(t[:], wqb2_d[:, h * 6 * 768:(h + 1) * 6 * 768])
            wqb_t.append(t)
        cosf_t = p_cs.tile([128, S], bf, tag="cs", name="cs")
        sinf_t = p_cs.tile([128, S], bf, tag="cs", name="cs")
        nc.scalar.dma_start(cosf_t[:], cosf_d[:, :])
        nc.gpsimd.dma_start(sinf_t[:], sinfsg_d[:, :])
        steps_t = p_msk.tile([128, 4 * 512], bf, tag="msk", name="msk")
        nc.scalar.dma_start(steps_t[:], steps_d[:, :])
        negeye_t = p_msk.tile([128, 128], bf, tag="ne", name="ne")
        nc.gpsimd.dma_start(negeye_t[:], negeye_d[:, :])

        def wqb_ap(k, c0, c1):
            return wqb_t[k // 6][:, (k % 6) * 768 + c0:(k % 6) * 768 + c1]

        ssq_g = []
        raw_q = []
        for g in range(2):
            ql_ps = ([pp_o.tile([128, SL], f32, tag="po", name="po")
                      for _ in range(4)] +
                     [pp_s.tile([128, SL], f32, tag="ps", name="ps")
                      for _ in range(2)])
            for k2 in range(K2):
                hs2 = p_hs.tile([128, 2 * SL], bf, tag="hs", name="hs")
                heng = nc.sync if k2 % 2 == 0 else nc.scalar
                heng.dma_start(hs2[:], hsT2_d[:, k2 * 512:(k2 + 1) * 512])
                wq = []
                for kk in range(2):
                    k = 2 * k2 + kk
                    w = p_w.tile([128, QLR // 2], bf, tag="wqa", name="wqa",
                                 bufs=6)
                    eng = nc.sync if kk == 0 else nc.scalar
                    eng.dma_start(
                        w[:], wqa_d[k * 128:(k + 1) * 128,
                                    g * (QLR // 2):(g + 1) * (QLR // 2)])
                    wq.append(w)
                for kk in range(2):
                    k = 2 * k2 + kk
                    hh = hs2[:, kk * SL:(kk + 1) * SL]
                    for mi in range(6):
                        nc.tensor.matmul(ql_ps[mi][:],
                                         wq[kk][:, mi * 128:(mi + 1) * 128],
                                         hh, start=(k == 0),
                                         stop=(k == KH - 1))
            ssq_gt = pp_sm.tile([1, SL], f32, tag="ps", name="ps")
            ssq_g.append(ssq_gt)
            for mi in range(6):
                r = p_raw.tile([128, SL], bf, tag="raw", name="raw")
                nc.scalar.activation(r[:], ql_ps[mi][:], AF.Copy)
                raw_q.append(r)
                sq = p_sq.tile([128, SL], bf, tag="sq", name="sq")
                nc.scalar.activation(sq[:], ql_ps[mi][:], AF.Square)
                nc.tensor.matmul(ssq_gt[:], ones_col[:], sq[:],
                                 start=(mi == 0), stop=(mi == 5))
        ssg0 = p_sml.tile([1, SL], f32, tag="sml", name="sml")
        nc.scalar.activation(ssg0[:], ssq_g[0][:], AF.Copy)
        ssum_q = p_sml.tile([1, SL], f32, tag="sml", name="sml")
        nc.vector.tensor_add(ssum_q[:], ssg0[:], ssq_g[1][:])
        t_q = p_sml.tile([1, SL], f32, tag="sml", name="sml")
        nc.scalar.activation(t_q[:], ssum_q[:], AF.Sqrt,
                             bias=eps_t[:], scale=1.0 / QLR)
        s_q = p_sml.tile([1, SL], f32, tag="sml", name="sml")
        nc.vector.reciprocal(s_q[:], t_q[:])
        bq_sb = p_bc.tile([128, 512], f32, tag="bc", name="bc")
        nc.gpsimd.partition_broadcast(bq_sb[:, :SL], s_q[:])
        for m in range(KQ):
            qn = p_scn.tile([128, SL], bf, tag="scn", name="scn")
            nc.vector.tensor_mul(qn[:], raw_q[m][:], bq_sb[:, :SL])
            nc.scalar.dma_start(cc2_in[m * 128:(m + 1) * 128, :], qn[:])

        nc.gpsimd.collective_compute(
            "AllGather", mybir.AluOpType.bypass,
            ins=[cc2_in[:]], outs=[cc2_out[:]],
            replica_groups=[list(range(NC_N))],
        )

        # ---------------- BACK: K/V expansion (needs AG1 only) -------------
        RPC = 512 // SL     # AG rank-blocks per 512-wide seq chunk
        g_engs = [nc.scalar, nc.gpsimd, nc.sync]
        g_rr = [0]

        def rr_eng():
            e = g_engs[g_rr[0] % 3]
            g_rr[0] += 1
            return e

        ckvg = {}
        for c in range(KC):
            for sc in range(SC):
                t = p_ckvg.tile([128, 512], bf, tag="ckvg", name="ckvg")
                for half in range(RPC):
                    r = RPC * sc + half
                    rr_eng().dma_start(
                        t[:, half * SL:(half + 1) * SL],
                        cc1_out[r * (KVLR + 128) + c * 128:
                                r * (KVLR + 128) + (c + 1) * 128, :])
                ckvg[(c, sc)] = t
        kpeg = {}
        for sc in range(SC):
            t = p_kpeg.tile([128, 512], bf, tag="kpeg", name="kpeg")
            for half in range(RPC):
                r = RPC * sc + half
                rr_eng().dma_start(
                    t[:, half * SL:(half + 1) * SL],
                    cc1_out[r * (KVLR + 128) + KVLR:
                            r * (KVLR + 128) + KVLR + 128, :])
            kpeg[sc] = t
        # w_o resident tiles: streamed after the gathers
        wo_t = []
        for h in range(5):
            t = p_wo.tile([128, 8 * 512], bf, tag="wo", name="wo")
            eng = g_engs[h % 2]
            eng.dma_start(t[:], wo2_d[:, h * 8 * 512:(h + 1) * 8 * 512])
            wo_t.append(t)

        def wo_ap(i, n):
            b = i * NW + n
            return wo_t[b // 8][:, (b % 8) * 512:(b % 8 + 1) * 512]

        # K^T per head: [DN, S]
        K_t = []
        for i in range(HPC):
            kt = p_K.tile([128, S], bf, tag="K", name="K")
            K_t.append(kt)
            for sc in range(SC):
                ps = pp_s.tile([128, 512], f32, tag="ps", name="ps")
                for c in range(KC):
                    nc.tensor.matmul(
                        ps[:],
                        kct_t[:, (i * KC + c) * 128:(i * KC + c + 1) * 128],
                        ckvg[(c, sc)][:],
                        start=(c == 0), stop=(c == KC - 1))
                nc.scalar.activation(kt[:, sc * 512:(sc + 1) * 512], ps[:],
                                     AF.Copy)

        # V natural: per seq-block [128, 4*DV]
        V_t = []
        for sb in range(SB):
            ps = pp_s.tile([128, 512], f32, tag="ps", name="ps")
            for c in range(KC):
                nc.tensor.matmul(
                    ps[:],
                    ckvg[(c, sb // 4)][:, (sb % 4) * 128:(sb % 4 + 1) * 128],
                    vcp_t[:, c * 512:(c + 1) * 512],
                    start=(c == 0), stop=(c == KC - 1))
            vt = p_V.tile([128, HPC * DV], bf, tag="V", name="V")
            nc.scalar.activation(vt[:], ps[:], AF.Copy)
            V_t.append(vt)

        # ---------------- BACK: per-chunk Q proj + attention + w_o ---------
        qg_engs = [nc.gpsimd, nc.scalar]

        def gather_qlg(sc):
            qlg = []
            for k in range(KQ):
                t = p_wk.tile([128, 512], bf, tag="wk", name="wk")
                for half in range(RPC):
                    r = RPC * sc + half
                    qg_engs[(2 * k + half) % 2].dma_start(
                        t[:, half * SL:(half + 1) * SL],
                        cc2_out[r * QLR + k * 128:r * QLR + (k + 1) * 128, :])
                qlg.append(t)
            return qlg

        qlg = gather_qlg(0)
        for sc in range(SC):
            # --- Q^T nope per head + pe packs (roped) ---
            qn_t = []
            for i in range(HPC):
                ps = pp_o.tile([128, 512], f32, tag="po", name="po")
                for k in range(KQ):
                    nc.tensor.matmul(ps[:], wqb_ap(k, i * 128, (i + 1) * 128),
                                     qlg[k][:], start=(k == 0),
                                     stop=(k == KQ - 1))
                qt = p_qn.tile([128, 512], bf, tag="Qn", name="Qn")
                nc.scalar.activation(qt[:], ps[:], AF.Copy)
                qn_t.append(qt)
            roped = []
            for pkt in range(2):
                ps_pe = pp_s.tile([128, 512], f32, tag="ps", name="ps")
                for k in range(KQ):
                    nc.tensor.matmul(
                        ps_pe[:],
                        wqb_ap(k, 512 + pkt * 128, 512 + (pkt + 1) * 128),
                        qlg[k][:], start=(k == 0), stop=(k == KQ - 1))
                u1 = p_f32.tile([128, 512], f32, tag="f32", name="f32")
                u2 = p_f32.tile([128, 512], f32, tag="f32", name="f32")
                rp = p_rope.tile([128, 512], bf, tag="rope", name="rope")
                rope_apply(rp[:], ps_pe, cosf_t, sinf_t, sc * 512, 512, u1, u2)
                roped.append(rp)

            # --- attention: score stream runs ahead; o/pacc consumes trail
            # by LAG iterations (cross-head), so the exp -> attn@v serial
            # chain is amortized over LAG j-steps and the PE stays busy ---
            LAG = 7
            nj = 4 * sc + 4
            o_ps_l, dinv_l = [], [None] * HPC
            pacc_l, pts_l = [], []
            pending = []

            def consume_one():
                ci, cj = pending.pop(0)
                nc.tensor.matmul(o_ps_l[ci][:],
                                 V_t[cj][:, ci * DV:(ci + 1) * DV],
                                 pts_l[ci][cj][:], start=(cj == 0),
                                 stop=(cj == nj - 1))
                if cj == 0:
                    nc.vector.tensor_copy(pacc_l[ci][:], pts_l[ci][cj][:])
                else:
                    nc.vector.tensor_add(pacc_l[ci][:], pacc_l[ci][:],
                                         pts_l[ci][cj][:])
                if cj == nj - 1:
                    d_ps = pp_sm.tile([1, 512], f32, tag="ps", name="ps")
                    nc.tensor.matmul(d_ps[:], ones_col[:], pacc_l[ci][:],
                                     start=True, stop=True)
                    dinv = p_sml.tile([1, 512], f32, tag="sml", name="sml")
                    nc.vector.reciprocal_approx_fast(dinv[:], d_ps[:])
                    dinv_l[ci] = dinv

            for i in range(HPC):
                pkt, hp = i // 2, i % 2
                o_ps = pp_o.tile([128, 512], f32, tag="po", name="po")
                o_ps_l.append(o_ps)
                pacc = p_pacc.tile([128, 512], bf, tag="pacc", name="pacc")
                pacc_l.append(pacc)
                pts_l.append([])
                for j in range(nj):
                    s_ps = pp_s.tile([128, 512], f32, tag="ps", name="ps")
                    nc.tensor.matmul(s_ps[:],
                                     K_t[i][:, j * 128:(j + 1) * 128],
                                     qn_t[i][:], start=True, stop=False)
                    if j >= 4 * sc:
                        p = j - 4 * sc
                        nc.tensor.matmul(
                            s_ps[:], negeye_t[:],
                            steps_t[:, p * 512:(p + 1) * 512],
                            start=False, stop=False)
                    nc.tensor.matmul(
                        s_ps[:],
                        kpeg[j // 4][hp * 64:(hp + 1) * 64,
                                     (j % 4) * 128:(j % 4 + 1) * 128],
                        roped[pkt][hp * 64:(hp + 1) * 64, :],
                        start=False, stop=True)
                    pt = p_wk.tile([128, 512], bf, tag="wk", name="wk")
                    nc.scalar.activation(pt[:], s_ps[:], AF.Exp)
                    pts_l[i].append(pt)
                    pending.append((i, j))
                    if len(pending) > LAG:
                        consume_one()
            while pending:
                consume_one()

            oT = []
            for i in range(HPC):
                bc_sb = p_bc.tile([128, 512], f32, tag="bc", name="bc")
                nc.gpsimd.partition_broadcast(bc_sb[:], dinv_l[i][:])
                ot = p_oT.tile([128, 512], bf, tag="oT", name="oT")
                nc.vector.tensor_mul(ot[:], o_ps_l[i][:], bc_sb[:])
                oT.append(ot)

            # prefetch next chunk's gathered q_lora during w_o
            if sc + 1 < SC:
                qlg = gather_qlg(sc + 1)

            # --- w_o partial for this seq chunk ---
            # sbl outer / paired n inner: two [128,512] psum groups stage
            # into one [128,1024] tile and ship with a single DMA
            for sbl in range(4):
                sb = sc * 4 + sbl
                for n2 in range(NW // 2):
                    ob = p_out.tile([128, 1024], bf, tag="outst",
                                    name="outst")
                    for half in range(2):
                        n = 2 * n2 + half
                        ps = pp_s.tile([128, 512], f32, tag="ps", name="ps")
                        for i in range(HPC):
                            nc.tensor.matmul(
                                ps[:], oT[i][:, sbl * 128:(sbl + 1) * 128],
                                wo_ap(i, n), start=(i == 0),
                                stop=(i == HPC - 1))
                        ceng = nc.scalar if half == 0 else nc.vector
                        if half == 0:
                            ceng.activation(ob[:, :512], ps[:], AF.Copy)
                        else:
                            ceng.tensor_copy(ob[:, 512:], ps[:])
                    nc.sync.dma_start(
                        out_d[sb * 128:(sb + 1) * 128,
                              n2 * 1024:(n2 + 1) * 1024], ob[:])

    nc.compile()
    return nc


def _prep_inputs(inputs):
    """Host-side sharding + weight folding. Returns in_maps (list of 8 dicts)."""
    BF = _bf16()

    hs = np.asarray(inputs['hidden_states'], np.float32)
    pos = np.asarray(inputs['positions'])
    w_qa = np.asarray(inputs['w_qa'], np.float32)
    q_a_ln_w = np.asarray(inputs['q_a_ln_w'], np.float32)
    w_qb = np.asarray(inputs['w_qb'], np.float32)
    w_kva = np.asarray(inputs['w_kva'], np.float32)
    kv_a_ln_w = np.asarray(inputs['kv_a_ln_w'], np.float32)
    kc = np.asarray(inputs['kc'], np.float32)
    vc = np.asarray(inputs['vc'], np.float32)
    w_o = np.asarray(inputs['w_o'], np.float32)

    perm = np.concatenate([np.arange(0, DR, 2), np.arange(1, DR, 2)])
    inv_freq = 1.0 / (ROPE_BASE ** (np.arange(0, DR, 2, dtype=np.float64) / DR))
    freqs = pos.astype(np.float64)[None, :] * inv_freq[:, None]     # [32, S]
    cosT = np.cos(freqs).astype(np.float32)
    sinT = np.sin(freqs).astype(np.float32)
    cos128 = np.tile(cosT, (4, 1)).astype(BF)                        # [128, S]
    sin128 = np.tile(sinT, (4, 1)).astype(np.float32)
    sgn = np.where((np.arange(128) % 64) < 32, -1.0, 1.0)[:, None]
    sinsg128 = (sin128 * sgn).astype(BF)

    scale = DQ ** -0.5
    w_qb_eff = ((w_qb * q_a_ln_w[:, None]) * scale).reshape(QLR, H, DQ)

    w_pe = w_kva[:, KVLR:][:, perm]
    wkvpe = np.concatenate([w_kva[:, :KVLR], w_pe, w_pe], 1)   # [HID, 640]
    K2h = (HID // 128) // 2
    wkvpe2 = wkvpe.reshape(K2h, 2, 128, 640).transpose(2, 0, 1, 3) \
        .reshape(128, K2h * 1280).astype(BF)

    kc_f = kc * kv_a_ln_w[None, None, :]
    vc_f = vc * kv_a_ln_w[None, :, None]

    # step tables: steps[r, p*512+q] = 1 if p*128+r > q else 0
    steps = np.zeros((128, 4 * 512), np.float32)
    rr = np.arange(128)[:, None]
    qq = np.arange(512)[None, :]
    for p in range(4):
        steps[:, p * 512:(p + 1) * 512] = (p * 128 + rr > qq)
    steps_b = steps.astype(BF)
    negeye = (-1e30 * np.eye(128, dtype=np.float32)).astype(BF)

    wqa_b = w_qa.astype(BF)

    K2 = (HID // 128) // 2
    NW = HID // 512

    in_maps = []
    for core in range(NC_N):
        rows = slice(core * SL, (core + 1) * SL)
        h0 = core * HPC

        hsT = np.ascontiguousarray(hs[rows].T)                   # [HID, SL]
        hsT2 = hsT.reshape(K2, 2, 128, SL).transpose(2, 0, 1, 3) \
            .reshape(128, K2 * 2 * SL)

        wqb_all = np.empty((QLR, 768), np.float32)
        for i in range(HPC):
            wqb_all[:, i * 128:(i + 1) * 128] = w_qb_eff[:, h0 + i, :DN]
        for pkt in range(2):
            a, b = h0 + 2 * pkt, h0 + 2 * pkt + 1
            pe_a = w_qb_eff[:, a, DN:][:, perm]
            pe_b = w_qb_eff[:, b, DN:][:, perm]
            wqb_all[:, 512 + pkt * 128:512 + pkt * 128 + 64] = pe_a
            wqb_all[:, 512 + pkt * 128 + 64:512 + (pkt + 1) * 128] = pe_b
        wqb2 = wqb_all.reshape(12, 128, 768).transpose(1, 0, 2) \
            .reshape(128, 12 * 768)

        kct = np.stack([kc_f[h0 + i].T[c * 128:(c + 1) * 128]
                        for i in range(HPC) for c in range(KVLR // 128)])
        kct2 = kct.transpose(1, 0, 2).reshape(128, -1)           # [128, 2048]

        vcp = np.concatenate([vc_f[h0 + i] for i in range(HPC)], 1)
        vcp2 = vcp.reshape(KVLR // 128, 128, HPC * DV) \
            .transpose(1, 0, 2).reshape(128, -1)                 # [128, 2048]

        wo_sh = w_o[h0 * DV:(h0 + HPC) * DV, :]                  # [512, HID]
        wo2 = wo_sh.reshape(HPC, 128, NW, 512).transpose(0, 2, 1, 3) \
            .reshape(HPC * NW, 128, 512).transpose(1, 0, 2) \
            .reshape(128, -1)                                    # [128, 20480]

        in_maps.append({
            "hsT2": hsT2.astype(BF),
            "wqa": wqa_b,
            "wkvpe2": wkvpe2,
            "cosl": np.ascontiguousarray(cos128[:, rows]),
            "sinlsg": np.ascontiguousarray(sinsg128[:, rows]),
            "cosf": cos128,
            "sinfsg": sinsg128,
            "wqb2": wqb2.astype(BF),
            "kct2": kct2.astype(BF),
            "vcp2": vcp2.astype(BF),
            "wo2": wo2.astype(BF),
            "steps": steps_b,
            "negeye": negeye,
        })
    return in_maps


def _get_program():
    if "nc" not in _CACHE:
        _CACHE["nc"] = _build_program()
    return _CACHE["nc"]


def run(inputs, trace=False, trace_kwargs=None):
    """Build (cached), run on 8 cores, return (output, BassKernelResults)."""
    from concourse.bass_utils import run_bass_kernel_spmd

    nc = _get_program()
    in_maps = _prep_inputs(inputs)
    res = run_bass_kernel_spmd(nc, in_maps, list(range(NC_N)),
                               trace=trace, **(trace_kwargs or {}))
    out = np.zeros((S, HID), np.float32)
    for r in res.results:
        out += np.asarray(r["out_partial"], dtype=np.float32)
    return out, res


def kernel(**inputs) -> np.ndarray:
    out, _ = run(inputs, trace=False)
    return out


# revision 31
# speedup vs baseline: 1.1058x; 1.0020x over previous
"""Trainium2 Bass kernel for a DeepseekV2 decoder-layer attention block
(MLA prefill, fp32 reference) distributed across 8 NeuronCores.

Strategy (single NEFF, SPMD on 8 cores):
  - Sequence-shard the shared projections: each core computes ckv / k_pe
    (RMS-normed / roped) then q_lora for its 256 rows of the sequence, in
    transposed layout; two on-device AllGathers replicate them. ckv goes
    first so its AllGather flies under the q_lora GEMM; the q_lora
    AllGather flies under the K/V expansion.
  - Head-shard the rest (4 heads per core): q_b projection + RoPE, kc/vc
    expansion, causal attention (scores computed transposed so the attn@v
    matmul needs no transposes), and a row-shard of w_o.
  - Each core emits a partial [S, HID] bf16 output; the host sums the 8
    partials (the output all-reduce) to produce the full result.

Perf structure (v3):
  - All weight/activation streams are host-packed into [128, N] layouts so
    every DMA is one large transfer, spread round-robin across the
    sync/scalar/gpsimd/vector DGE rings (the per-DMA ~600ns issue cost made
    the v2 front end ring-bound).
  - Attention is software-pipelined (lookahead 2); the causal mask is
    applied as a third matmul into the score PSUM group (-1e30 * step),
    so the exp -> attn@v chain has no vector-engine hop.
  - Softmax denominators: bf16 P-tile running sum on the vector engine,
    one ones-matmul per (head, chunk), reciprocal_approx_fast.
  - RoPE rotation via partition-offset vector ops with sign-folded sin.
  - w_o resident; output staged bf16 through the scalar engine.
"""

import numpy as np

S, HID, H = 2048, 5120, 32
QLR, KVLR = 1536, 512
DN, DR, DV = 128, 64, 128
DQ = DN + DR
NC_N = 8
HPC = H // NC_N          # heads per core
SL = S // NC_N           # sequence rows per core (front end)
ROPE_BASE, EPS = 10000.0, 1e-6

_CACHE = {}


def _bf16():
    import ml_dtypes
    return np.dtype(ml_dtypes.bfloat16)


def _build_program():
    import concourse.bass as bass
    import concourse.tile as tile
    from concourse import bacc, mybir
    from contextlib import ExitStack

    f32 = mybir.dt.float32
    bf = mybir.dt.bfloat16
    AF = mybir.ActivationFunctionType

    nc = bacc.Bacc("TRN2", target_bir_lowering=False, debug=False,
                   num_devices=NC_N)

    def din(name, shape, dt=bf):
        return nc.dram_tensor(name, list(shape), dt, kind="ExternalInput").ap()

    KH = HID // 128       # 40 k-chunks of the model dim
    K2 = KH // 2          # 20 double-chunks
    KQ = QLR // 128       # 12 chunks of the q-lora dim
    KC = KVLR // 128      # 4 chunks of the kv-lora dim
    SC = S // 512         # 4 sequence chunks of 512
    SB = S // 128         # 16 sequence blocks of 128
    NW = HID // 512       # 10 w_o column chunks

    hsT2_d = din("hsT2", (128, K2 * 2 * SL))    # [p, k2*512+half*256+c]
    wqa_d = din("wqa", (HID, QLR))
    wkvpe2_d = din("wkvpe2", (128, (HID // 256) * 1280))  # packed pairs
    cosl_d = din("cosl", (128, SL))
    sinlsg_d = din("sinlsg", (128, SL))         # sign-folded sin
    cosf_d = din("cosf", (128, S))
    sinfsg_d = din("sinfsg", (128, S))
    wqb2_d = din("wqb2", (128, KQ * 768))       # [p, k*768 + col]
    kct2_d = din("kct2", (128, HPC * KC * 128))  # [p, (i*4+c)*128 + d]
    vcp2_d = din("vcp2", (128, KC * HPC * DV))  # [p, c*512 + col]
    wo2_d = din("wo2", (128, HPC * NW * 512))   # [p, (i*NW+n)*512 + col]
    steps_d = din("steps", (128, 4 * 512))      # keep01: [p*128+r <= q]
    out_d = nc.dram_tensor("out_partial", [S, HID], bf,
                           kind="ExternalOutput").ap()

    cc1_in = nc.dram_tensor("cc1_in", [KVLR + 128, SL], bf).ap()
    cc1_out = nc.dram_tensor("cc1_out", [NC_N * (KVLR + 128), SL], bf,
                             addr_space="Shared").ap()
    cc2_in = nc.dram_tensor("cc2_in", [QLR, SL], bf).ap()
    cc2_out = nc.dram_tensor("cc2_out", [NC_N * QLR, SL], bf,
                             addr_space="Shared").ap()

    with tile.TileContext(nc) as tc, ExitStack() as ctx:
        def pool(name, bufs):
            return ctx.enter_context(tc.tile_pool(name=name, bufs=bufs))

        p_hs = pool("hs", 4)
        p_w = pool("wstr", 3)
        p_raw = pool("raw", 12)
        p_sq = pool("sqt", 1)
        p_scn = pool("scn", 2)
        p_sml = pool("sml", 4)
        p_one = pool("ones", 2)
        p_cs = pool("cs", 2)
        p_csl = pool("csl", 2)
        p_bc = pool("bc", 2)
        p_kc = pool("kc", 1)
        p_vc = pool("vc", 1)
        p_wqb = pool("wqb", 2)
        p_wo = pool("wo", 5)
        p_msk = pool("msk", 1)
        p_kpeg = pool("kpeg", 4)
        p_K = pool("Kt", 4)
        p_V = pool("Vt", 16)
        p_qn = pool("Qn", 4)
        p_rope = pool("rope", 2)
        p_f32 = pool("fr32", 2)
        p_ckvg = pool("ckvg", 16)
        p_wk = pool("wk", 14)       # shared ring: qlg -> P tiles
        p_pacc = pool("pacc", 2)
        p_oT = pool("oT", 4)
        p_out = pool("outst", 2)

        pp_o = ctx.enter_context(
            tc.tile_pool(name="ppo", bufs=4, space="PSUM"))
        pp_s = ctx.enter_context(
            tc.tile_pool(name="pps", bufs=4, space="PSUM"))
        pp_sm = pp_s

        ones_col = p_one.tile([128, 1], bf)       # lhsT for column sums
        nc.vector.memset(ones_col[:], 1.0)
        eps_t = p_one.tile([1, 1], f32, tag="eps", name="eps")
        nc.vector.memset(eps_t[:], EPS)

        def rope_apply(dst, src_ps, cos_t, sin_t, c0, width, u1, u2):
            """dst(bf16) = src*cos + rot(src)*sin_signed, via partition-offset
            muls. src_ps is a [128, width] f32 PSUM pack of 64-dim halves."""
            nc.vector.tensor_mul(u1[:, :width], src_ps[:],
                                 cos_t[:, c0:c0 + width])
            for blk in (0, 64):
                nc.vector.tensor_mul(
                    u2[blk:blk + 32, :width],
                    src_ps[blk + 32:blk + 64, :],
                    sin_t[blk:blk + 32, c0:c0 + width])
                nc.vector.tensor_mul(
                    u2[blk + 32:blk + 64, :width],
                    src_ps[blk:blk + 32, :],
                    sin_t[blk + 32:blk + 64, c0:c0 + width])
            nc.vector.tensor_add(dst, u1[:, :width], u2[:, :width])

        # ---------------- FRONT 1: ckv + k_pe pass (then AG1) --------------
        ckv_ps = [pp_o.tile([128, SL], f32, tag="po", name="po")
                  for _ in range(KC)]
        pe_ps = pp_s.tile([128, SL], f32, tag="ps", name="ps")
        for k2 in range(K2):
            hs2 = p_hs.tile([128, 2 * SL], bf, tag="hs", name="hs")
            nc.sync.dma_start(hs2[:], hsT2_d[:, k2 * 512:(k2 + 1) * 512])
            wv = p_w.tile([128, 1280], bf, tag="wkv", name="wkv", bufs=3)
            nc.scalar.dma_start(wv[:], wkvpe2_d[:, k2 * 1280:(k2 + 1) * 1280])
            for kk in range(2):
                k = 2 * k2 + kk
                hh = hs2[:, kk * SL:(kk + 1) * SL]
                wk0 = kk * 640
                for c in range(KC):
                    nc.tensor.matmul(ckv_ps[c][:],
                                     wv[:, wk0 + c * 128:wk0 + (c + 1) * 128],
                                     hh, start=(k == 0), stop=(k == KH - 1))
                nc.tensor.matmul(pe_ps[:], wv[:, wk0 + KVLR:wk0 + KVLR + 128],
                                 hh,
                                 start=(k == 0), stop=(k == KH - 1))
        ssq_kv = pp_sm.tile([1, SL], f32, tag="ps", name="ps")
        raw_kv = []
        for c in range(KC):
            r = p_raw.tile([128, SL], bf, tag="raw", name="raw")
            nc.scalar.activation(r[:], ckv_ps[c][:], AF.Copy)
            raw_kv.append(r)
            sq = p_sq.tile([128, SL], bf, tag="sq", name="sq")
            nc.scalar.activation(sq[:], ckv_ps[c][:], AF.Square)
            nc.tensor.matmul(ssq_kv[:], ones_col[:], sq[:],
                             start=(c == 0), stop=(c == KC - 1))
        t_kv = p_sml.tile([1, SL], f32, tag="sml", name="sml")
        nc.scalar.activation(t_kv[:], ssq_kv[:], AF.Sqrt,
                             bias=eps_t[:], scale=1.0 / KVLR)
        s_kv = p_sml.tile([1, SL], f32, tag="sml", name="sml")
        nc.vector.reciprocal(s_kv[:], t_kv[:])
        bkv_sb = p_bc.tile([128, 512], f32, tag="bc", name="bc")
        nc.gpsimd.partition_broadcast(bkv_sb[:, :SL], s_kv[:])
        for c in range(KC):
            cn = p_scn.tile([128, SL], bf, tag="scn", name="scn")
            nc.vector.tensor_mul(cn[:], raw_kv[c][:], bkv_sb[:, :SL])
            nc.scalar.dma_start(cc1_in[c * 128:(c + 1) * 128, :], cn[:])
        # rope k_pe (partition-offset rotation, sign folded into sin table)
        cosl_t = p_csl.tile([128, SL], bf, tag="csl", name="csl")
        sinl_t = p_csl.tile([128, SL], bf, tag="csl", name="csl")
        nc.sync.dma_start(cosl_t[:], cosl_d[:, :])
        nc.sync.dma_start(sinl_t[:], sinlsg_d[:, :])
        u1f = p_f32.tile([128, 512], f32, tag="f32", name="f32")
        u2f = p_f32.tile([128, 512], f32, tag="f32", name="f32")
        kpe_n = p_scn.tile([128, SL], bf, tag="scn", name="scn")
        rope_apply(kpe_n[:], pe_ps, cosl_t, sinl_t, 0, SL, u1f, u2f)
        nc.scalar.dma_start(cc1_in[KVLR:KVLR + 128, :], kpe_n[:])

        nc.gpsimd.collective_compute(
            "AllGather", mybir.AluOpType.bypass,
            ins=[cc1_in[:]], outs=[cc1_out[:]],
            replica_groups=[list(range(NC_N))],
        )

        # ---------------- FRONT 2: q_lora pass (then AG2) ------------------
        # resident back-end weights stream on the scalar/gpsimd rings
        kct_t = p_kc.tile([128, HPC * KC * 128], bf, tag="kc", name="kc")
        nc.scalar.dma_start(kct_t[:], kct2_d[:, :])
        vcp_t = p_vc.tile([128, KC * HPC * DV], bf, tag="vc", name="vc")
        nc.gpsimd.dma_start(vcp_t[:], vcp2_d[:, :])
        wqb_t = []
        for h in range(2):
            t = p_wqb.tile([128, 6 * 768], bf, tag="wqb", name="wqb")
            eng = nc.scalar if h == 0 else nc.gpsimd
            eng.dma_start(t[:], wqb2_d[:, h * 6 * 768:(h + 1) * 6 * 768])
            wqb_t.append(t)
        cosf_t = p_cs.tile([128, S], bf, tag="cs", name="cs")
        sinf_t = p_cs.tile([128, S], bf, tag="cs", name="cs")
        nc.scalar.dma_start(cosf_t[:], cosf_d[:, :])
        nc.gpsimd.dma_start(sinf_t[:], sinfsg_d[:, :])
        steps_t = p_msk.tile([128, 4 * 512], bf, tag="msk", name="msk")
        nc.scalar.dma_start(steps_t[:], steps_d[:, :])

        def wqb_ap(k, c0, c1):
            return wqb_t[k // 6][:, (k % 6) * 768 + c0:(k % 6) * 768 + c1]

        ssq_g = []
        raw_q = []
        for g in range(2):
            ql_ps = ([pp_o.tile([128, SL], f32, tag="po", name="po")
                      for _ in range(4)] +
                     [pp_s.tile([128, SL], f32, tag="ps", name="ps")
                      for _ in range(2)])
            for k2 in range(K2):
                hs2 = p_hs.tile([128, 2 * SL], bf, tag="hs", name="hs")
                heng = nc.sync if k2 % 2 == 0 else nc.scalar
                heng.dma_start(hs2[:], hsT2_d[:, k2 * 512:(k2 + 1) * 512])
                wq = []
                for kk in range(2):
                    k = 2 * k2 + kk
                    w = p_w.tile([128, QLR // 2], bf, tag="wqa", name="wqa",
                                 bufs=6)
                    eng = nc.sync if kk == 0 else nc.scalar
                    eng.dma_start(
                        w[:], wqa_d[k * 128:(k + 1) * 128,
                                    g * (QLR // 2):(g + 1) * (QLR // 2)])
                    wq.append(w)
                for kk in range(2):
                    k = 2 * k2 + kk
                    hh = hs2[:, kk * SL:(kk + 1) * SL]
                    for mi in range(6):
                        nc.tensor.matmul(ql_ps[mi][:],
                                         wq[kk][:, mi * 128:(mi + 1) * 128],
                                         hh, start=(k == 0),
                                         stop=(k == KH - 1))
            ssq_gt = pp_sm.tile([1, SL], f32, tag="ps", name="ps")
            ssq_g.append(ssq_gt)
            for mi in range(6):
                r = p_raw.tile([128, SL], bf, tag="raw", name="raw")
                nc.scalar.activation(r[:], ql_ps[mi][:], AF.Copy)
                raw_q.append(r)
                sq = p_sq.tile([128, SL], bf, tag="sq", name="sq")
                nc.scalar.activation(sq[:], ql_ps[mi][:], AF.Square)
                nc.tensor.matmul(ssq_gt[:], ones_col[:], sq[:],
                                 start=(mi == 0), stop=(mi == 5))
        ssg0 = p_sml.tile([1, SL], f32, tag="sml", name="sml")
        nc.scalar.activation(ssg0[:], ssq_g[0][:], AF.Copy)
        ssum_q = p_sml.tile([1, SL], f32, tag="sml", name="sml")
        nc.vector.tensor_add(ssum_q[:], ssg0[:], ssq_g[1][:])
        t_q = p_sml.tile([1, SL], f32, tag="sml", name="sml")
        nc.scalar.activation(t_q[:], ssum_q[:], AF.Sqrt,
                             bias=eps_t[:], scale=1.0 / QLR)
        s_q = p_sml.tile([1, SL], f32, tag="sml", name="sml")
        nc.vector.reciprocal(s_q[:], t_q[:])
        bq_sb = p_bc.tile([128, 512], f32, tag="bc", name="bc")
        nc.gpsimd.partition_broadcast(bq_sb[:, :SL], s_q[:])
        for m in range(KQ):
            qn = p_scn.tile([128, SL], bf, tag="scn", name="scn")
            nc.vector.tensor_mul(qn[:], raw_q[m][:], bq_sb[:, :SL])
            nc.scalar.dma_start(cc2_in[m * 128:(m + 1) * 128, :], qn[:])

        nc.gpsimd.collective_compute(
            "AllGather", mybir.AluOpType.bypass,
            ins=[cc2_in[:]], outs=[cc2_out[:]],
            replica_groups=[list(range(NC_N))],
        )

        # ---------------- BACK: K/V expansion (needs AG1 only) -------------
        RPC = 512 // SL     # AG rank-blocks per 512-wide seq chunk
        g_engs = [nc.scalar, nc.gpsimd, nc.sync]
        g_rr = [0]

        def rr_eng():
            e = g_engs[g_rr[0] % 3]
            g_rr[0] += 1
            return e

        ckvg = {}
        for c in range(KC):
            for sc in range(SC):
                t = p_ckvg.tile([128, 512], bf, tag="ckvg", name="ckvg")
                for half in range(RPC):
                    r = RPC * sc + half
                    rr_eng().dma_start(
                        t[:, half * SL:(half + 1) * SL],
                        cc1_out[r * (KVLR + 128) + c * 128:
                                r * (KVLR + 128) + (c + 1) * 128, :])
                ckvg[(c, sc)] = t
        kpeg = {}
        for sc in range(SC):
            t = p_kpeg.tile([128, 512], bf, tag="kpeg", name="kpeg")
            for half in range(RPC):
                r = RPC * sc + half
                rr_eng().dma_start(
                    t[:, half * SL:(half + 1) * SL],
                    cc1_out[r * (KVLR + 128) + KVLR:
                            r * (KVLR + 128) + KVLR + 128, :])
            kpeg[sc] = t
        # w_o resident tiles: streamed after the gathers
        wo_t = []
        for h in range(5):
            t = p_wo.tile([128, 8 * 512], bf, tag="wo", name="wo")
            eng = g_engs[h % 2]
            eng.dma_start(t[:], wo2_d[:, h * 8 * 512:(h + 1) * 8 * 512])
            wo_t.append(t)

        def wo_ap(i, n):
            b = i * NW + n
            return wo_t[b // 8][:, (b % 8) * 512:(b % 8 + 1) * 512]

        # K^T per head: [DN, S]
        K_t = []
        for i in range(HPC):
            kt = p_K.tile([128, S], bf, tag="K", name="K")
            K_t.append(kt)
            for sc in range(SC):
                ps = pp_s.tile([128, 512], f32, tag="ps", name="ps")
                for c in range(KC):
                    nc.tensor.matmul(
                        ps[:],
                        kct_t[:, (i * KC + c) * 128:(i * KC + c + 1) * 128],
                        ckvg[(c, sc)][:],
                        start=(c == 0), stop=(c == KC - 1))
                nc.scalar.activation(kt[:, sc * 512:(sc + 1) * 512], ps[:],
                                     AF.Copy)

        # V natural: per seq-block [128, 4*DV]
        V_t = []
        for sb in range(SB):
            ps = pp_s.tile([128, 512], f32, tag="ps", name="ps")
            for c in range(KC):
                nc.tensor.matmul(
                    ps[:],
                    ckvg[(c, sb // 4)][:, (sb % 4) * 128:(sb % 4 + 1) * 128],
                    vcp_t[:, c * 512:(c + 1) * 512],
                    start=(c == 0), stop=(c == KC - 1))
            vt = p_V.tile([128, HPC * DV], bf, tag="V", name="V")
            nc.scalar.activation(vt[:], ps[:], AF.Copy)
            V_t.append(vt)

        # ---------------- BACK: per-chunk Q proj + attention + w_o ---------
        qg_engs = [nc.gpsimd, nc.scalar]

        def gather_qlg(sc):
            qlg = []
            for k in range(KQ):
                t = p_wk.tile([128, 512], bf, tag="wk", name="wk")
                for half in range(RPC):
                    r = RPC * sc + half
                    qg_engs[(2 * k + half) % 2].dma_start(
                        t[:, half * SL:(half + 1) * SL],
                        cc2_out[r * QLR + k * 128:r * QLR + (k + 1) * 128, :])
                qlg.append(t)
            return qlg

        qlg = gather_qlg(0)
        for sc in range(SC):
            # --- Q^T nope per head + pe packs (roped) ---
            qn_t = []
            for i in range(HPC):
                ps = pp_o.tile([128, 512], f32, tag="po", name="po")
                for k in range(KQ):
                    nc.tensor.matmul(ps[:], wqb_ap(k, i * 128, (i + 1) * 128),
                                     qlg[k][:], start=(k == 0),
                                     stop=(k == KQ - 1))
                qt = p_qn.tile([128, 512], bf, tag="Qn", name="Qn")
                nc.scalar.activation(qt[:], ps[:], AF.Copy)
                qn_t.append(qt)
            roped = []
            for pkt in range(2):
                ps_pe = pp_s.tile([128, 512], f32, tag="ps", name="ps")
                for k in range(KQ):
                    nc.tensor.matmul(
                        ps_pe[:],
                        wqb_ap(k, 512 + pkt * 128, 512 + (pkt + 1) * 128),
                        qlg[k][:], start=(k == 0), stop=(k == KQ - 1))
                u1 = p_f32.tile([128, 512], f32, tag="f32", name="f32")
                u2 = p_f32.tile([128, 512], f32, tag="f32", name="f32")
                rp = p_rope.tile([128, 512], bf, tag="rope", name="rope")
                rope_apply(rp[:], ps_pe, cosf_t, sinf_t, sc * 512, 512, u1, u2)
                roped.append(rp)

            # --- attention: score stream runs ahead; o/pacc consumes trail
            # by LAG iterations (cross-head), so the exp -> attn@v serial
            # chain is amortized over LAG j-steps and the PE stays busy ---
            LAG = 9
            nj = 4 * sc + 4
            o_ps_l, dinv_l = [], [None] * HPC
            pacc_l, pts_l = [], []
            pending = []

            def consume_one():
                ci, cj = pending.pop(0)
                nc.tensor.matmul(o_ps_l[ci][:],
                                 V_t[cj][:, ci * DV:(ci + 1) * DV],
                                 pts_l[ci][cj][:], start=(cj == 0),
                                 stop=(cj == nj - 1))
                if cj == 0:
                    nc.vector.tensor_copy(pacc_l[ci][:], pts_l[ci][cj][:])
                else:
                    nc.vector.tensor_add(pacc_l[ci][:], pacc_l[ci][:],
                                         pts_l[ci][cj][:])
                if cj == nj - 1:
                    d_ps = pp_sm.tile([1, 512], f32, tag="ps", name="ps")
                    nc.tensor.matmul(d_ps[:], ones_col[:], pacc_l[ci][:],
                                     start=True, stop=True)
                    dinv = p_sml.tile([1, 512], f32, tag="sml", name="sml")
                    nc.vector.reciprocal_approx_fast(dinv[:], d_ps[:])
                    dinv_l[ci] = dinv

            for i in range(HPC):
                pkt, hp = i // 2, i % 2
                o_ps = pp_o.tile([128, 512], f32, tag="po", name="po")
                o_ps_l.append(o_ps)
                pacc = p_pacc.tile([128, 512], bf, tag="pacc", name="pacc")
                pacc_l.append(pacc)
                pts_l.append([])
                for j in range(nj):
                    s_ps = pp_s.tile([128, 512], f32, tag="ps", name="ps")
                    nc.tensor.matmul(s_ps[:],
                                     K_t[i][:, j * 128:(j + 1) * 128],
                                     qn_t[i][:], start=True, stop=False)
                    nc.tensor.matmul(
                        s_ps[:],
                        kpeg[j // 4][hp * 64:(hp + 1) * 64,
                                     (j % 4) * 128:(j % 4 + 1) * 128],
                        roped[pkt][hp * 64:(hp + 1) * 64, :],
                        start=False, stop=True)
                    if j >= 4 * sc:
                        # masked diagonal: exp then 0/1 keep-multiply on the
                        # vector engine — with the LAG consume queue this
                        # hop has ~LAG iterations of slack off the PE chain
                        p = j - 4 * sc
                        pr = p_wk.tile([128, 512], bf, tag="wk", name="wk")
                        nc.scalar.activation(pr[:], s_ps[:], AF.Exp)
                        pt = p_wk.tile([128, 512], bf, tag="wk", name="wk")
                        nc.vector.tensor_mul(pt[:], pr[:],
                                             steps_t[:, p * 512:(p + 1) * 512])
                    else:
                        pt = p_wk.tile([128, 512], bf, tag="wk", name="wk")
                        nc.scalar.activation(pt[:], s_ps[:], AF.Exp)
                    pts_l[i].append(pt)
                    pending.append((i, j))
                    if len(pending) > LAG:
                        consume_one()
            while pending:
                consume_one()

            oT = []
            for i in range(HPC):
                bc_sb = p_bc.tile([128, 512], f32, tag="bc", name="bc")
                nc.gpsimd.partition_broadcast(bc_sb[:], dinv_l[i][:])
                ot = p_oT.tile([128, 512], bf, tag="oT", name="oT")
                nc.vector.tensor_mul(ot[:], o_ps_l[i][:], bc_sb[:])
                oT.append(ot)

            # prefetch next chunk's gathered q_lora during w_o
            if sc + 1 < SC:
                qlg = gather_qlg(sc + 1)

            # --- w_o partial for this seq chunk ---
            # sbl outer / paired n inner: two [128,512] psum groups stage
            # into one [128,1024] tile and ship with a single DMA
            for sbl in range(4):
                sb = sc * 4 + sbl
                for n2 in range(NW // 2):
                    ob = p_out.tile([128, 1024], bf, tag="outst",
                                    name="outst")
                    for half in range(2):
                        n = 2 * n2 + half
                        ps = pp_s.tile([128, 512], f32, tag="ps", name="ps")
                        for i in range(HPC):
                            nc.tensor.matmul(
                                ps[:], oT[i][:, sbl * 128:(sbl + 1) * 128],
                                wo_ap(i, n), start=(i == 0),
                                stop=(i == HPC - 1))
                        ceng = nc.scalar if half == 0 else nc.vector
                        if half == 0:
                            ceng.activation(ob[:, :512], ps[:], AF.Copy)
                        else:
                            ceng.tensor_copy(ob[:, 512:], ps[:])
                    nc.sync.dma_start(
                        out_d[sb * 128:(sb + 1) * 128,
                              n2 * 1024:(n2 + 1) * 1024], ob[:])

    nc.compile()
    return nc


def _prep_inputs(inputs):
    """Host-side sharding + weight folding. Returns in_maps (list of 8 dicts)."""
    BF = _bf16()

    hs = np.asarray(inputs['hidden_states'], np.float32)
    pos = np.asarray(inputs['positions'])
    w_qa = np.asarray(inputs['w_qa'], np.float32)
    q_a_ln_w = np.asarray(inputs['q_a_ln_w'], np.float32)
    w_qb = np.asarray(inputs['w_qb'], np.float32)
    w_kva = np.asarray(inputs['w_kva'], np.float32)
    kv_a_ln_w = np.asarray(inputs['kv_a_ln_w'], np.float32)
    kc = np.asarray(inputs['kc'], np.float32)
    vc = np.asarray(inputs['vc'], np.float32)
    w_o = np.asarray(inputs['w_o'], np.float32)

    perm = np.concatenate([np.arange(0, DR, 2), np.arange(1, DR, 2)])
    inv_freq = 1.0 / (ROPE_BASE ** (np.arange(0, DR, 2, dtype=np.float64) / DR))
    freqs = pos.astype(np.float64)[None, :] * inv_freq[:, None]     # [32, S]
    cosT = np.cos(freqs).astype(np.float32)
    sinT = np.sin(freqs).astype(np.float32)
    cos128 = np.tile(cosT, (4, 1)).astype(BF)                        # [128, S]
    sin128 = np.tile(sinT, (4, 1)).astype(np.float32)
    sgn = np.where((np.arange(128) % 64) < 32, -1.0, 1.0)[:, None]
    sinsg128 = (sin128 * sgn).astype(BF)

    scale = DQ ** -0.5
    w_qb_eff = ((w_qb * q_a_ln_w[:, None]) * scale).reshape(QLR, H, DQ)

    w_pe = w_kva[:, KVLR:][:, perm]
    wkvpe = np.concatenate([w_kva[:, :KVLR], w_pe, w_pe], 1)   # [HID, 640]
    K2h = (HID // 128) // 2
    wkvpe2 = wkvpe.reshape(K2h, 2, 128, 640).transpose(2, 0, 1, 3) \
        .reshape(128, K2h * 1280).astype(BF)

    kc_f = kc * kv_a_ln_w[None, None, :]
    vc_f = vc * kv_a_ln_w[None, :, None]

    # keep tables: steps[r, p*512+q] = 0 if p*128+r > q else 1 (causal keep)
    steps = np.zeros((128, 4 * 512), np.float32)
    rr = np.arange(128)[:, None]
    qq = np.arange(512)[None, :]
    for p in range(4):
        steps[:, p * 512:(p + 1) * 512] = (p * 128 + rr <= qq)
    steps_b = steps.astype(BF)

    wqa_b = w_qa.astype(BF)

    K2 = (HID // 128) // 2
    NW = HID // 512

    in_maps = []
    for core in range(NC_N):
        rows = slice(core * SL, (core + 1) * SL)
        h0 = core * HPC

        hsT = np.ascontiguousarray(hs[rows].T)                   # [HID, SL]
        hsT2 = hsT.reshape(K2, 2, 128, SL).transpose(2, 0, 1, 3) \
            .reshape(128, K2 * 2 * SL)

        wqb_all = np.empty((QLR, 768), np.float32)
        for i in range(HPC):
            wqb_all[:, i * 128:(i + 1) * 128] = w_qb_eff[:, h0 + i, :DN]
        for pkt in range(2):
            a, b = h0 + 2 * pkt, h0 + 2 * pkt + 1
            pe_a = w_qb_eff[:, a, DN:][:, perm]
            pe_b = w_qb_eff[:, b, DN:][:, perm]
            wqb_all[:, 512 + pkt * 128:512 + pkt * 128 + 64] = pe_a
            wqb_all[:, 512 + pkt * 128 + 64:512 + (pkt + 1) * 128] = pe_b
        wqb2 = wqb_all.reshape(12, 128, 768).transpose(1, 0, 2) \
            .reshape(128, 12 * 768)

        kct = np.stack([kc_f[h0 + i].T[c * 128:(c + 1) * 128]
                        for i in range(HPC) for c in range(KVLR // 128)])
        kct2 = kct.transpose(1, 0, 2).reshape(128, -1)           # [128, 2048]

        vcp = np.concatenate([vc_f[h0 + i] for i in range(HPC)], 1)
        vcp2 = vcp.reshape(KVLR // 128, 128, HPC * DV) \
            .transpose(1, 0, 2).reshape(128, -1)                 # [128, 2048]

        wo_sh = w_o[h0 * DV:(h0 + HPC) * DV, :]                  # [512, HID]
        wo2 = wo_sh.reshape(HPC, 128, NW, 512).transpose(0, 2, 1, 3) \
            .reshape(HPC * NW, 128, 512).transpose(1, 0, 2) \
            .reshape(128, -1)                                    # [128, 20480]

        in_maps.append({
            "hsT2": hsT2.astype(BF),
            "wqa": wqa_b,
            "wkvpe2": wkvpe2,
            "cosl": np.ascontiguousarray(cos128[:, rows]),
            "sinlsg": np.ascontiguousarray(sinsg128[:, rows]),
            "cosf": cos128,
            "sinfsg": sinsg128,
            "wqb2": wqb2.astype(BF),
            "kct2": kct2.astype(BF),
            "vcp2": vcp2.astype(BF),
            "wo2": wo2.astype(BF),
            "steps": steps_b,
        })
    return in_maps


def _get_program():
    if "nc" not in _CACHE:
        _CACHE["nc"] = _build_program()
    return _CACHE["nc"]


def run(inputs, trace=False, trace_kwargs=None):
    """Build (cached), run on 8 cores, return (output, BassKernelResults)."""
    from concourse.bass_utils import run_bass_kernel_spmd

    nc = _get_program()
    in_maps = _prep_inputs(inputs)
    res = run_bass_kernel_spmd(nc, in_maps, list(range(NC_N)),
                               trace=trace, **(trace_kwargs or {}))
    out = np.zeros((S, HID), np.float32)
    for r in res.results:
        out += np.asarray(r["out_partial"], dtype=np.float32)
    return out, res


def kernel(**inputs) -> np.ndarray:
    out, _ = run(inputs, trace=False)
    return out


# revision 33
# speedup vs baseline: 1.1084x; 1.0024x over previous
"""Trainium2 Bass kernel for a DeepseekV2 decoder-layer attention block
(MLA prefill, fp32 reference) distributed across 8 NeuronCores.

Strategy (single NEFF, SPMD on 8 cores):
  - Sequence-shard the shared projections: each core computes ckv / k_pe
    (RMS-normed / roped) then q_lora for its 256 rows of the sequence, in
    transposed layout; two on-device AllGathers replicate them. ckv goes
    first so its AllGather flies under the q_lora GEMM; the q_lora
    AllGather flies under the K/V expansion.
  - Head-shard the rest (4 heads per core): q_b projection + RoPE, kc/vc
    expansion, causal attention (scores computed transposed so the attn@v
    matmul needs no transposes), and a row-shard of w_o.
  - Each core emits a partial [S, HID] bf16 output; the host sums the 8
    partials (the output all-reduce) to produce the full result.

Perf structure (v3):
  - All weight/activation streams are host-packed into [128, N] layouts so
    every DMA is one large transfer, spread round-robin across the
    sync/scalar/gpsimd/vector DGE rings (the per-DMA ~600ns issue cost made
    the v2 front end ring-bound).
  - Attention is software-pipelined (lookahead 2); the causal mask is
    applied as a third matmul into the score PSUM group (-1e30 * step),
    so the exp -> attn@v chain has no vector-engine hop.
  - Softmax denominators: bf16 P-tile running sum on the vector engine,
    one ones-matmul per (head, chunk), reciprocal_approx_fast.
  - RoPE rotation via partition-offset vector ops with sign-folded sin.
  - w_o resident; output staged bf16 through the scalar engine.
"""

import numpy as np

S, HID, H = 2048, 5120, 32
QLR, KVLR = 1536, 512
DN, DR, DV = 128, 64, 128
DQ = DN + DR
NC_N = 8
HPC = H // NC_N          # heads per core
SL = S // NC_N           # sequence rows per core (front end)
ROPE_BASE, EPS = 10000.0, 1e-6

_CACHE = {}


def _bf16():
    import ml_dtypes
    return np.dtype(ml_dtypes.bfloat16)


def _build_program():
    import concourse.bass as bass
    import concourse.tile as tile
    from concourse import bacc, mybir
    from contextlib import ExitStack

    f32 = mybir.dt.float32
    bf = mybir.dt.bfloat16
    AF = mybir.ActivationFunctionType

    nc = bacc.Bacc("TRN2", target_bir_lowering=False, debug=False,
                   num_devices=NC_N)

    def din(name, shape, dt=bf):
        return nc.dram_tensor(name, list(shape), dt, kind="ExternalInput").ap()

    KH = HID // 128       # 40 k-chunks of the model dim
    K2 = KH // 2          # 20 double-chunks
    KQ = QLR // 128       # 12 chunks of the q-lora dim
    KC = KVLR // 128      # 4 chunks of the kv-lora dim
    SC = S // 512         # 4 sequence chunks of 512
    SB = S // 128         # 16 sequence blocks of 128
    NW = HID // 512       # 10 w_o column chunks

    hsT2_d = din("hsT2", (128, K2 * 2 * SL))    # [p, k2*512+half*256+c]
    wqa_d = din("wqa", (HID, QLR))
    wkvpe2_d = din("wkvpe2", (128, (HID // 256) * 1280))  # packed pairs
    cosl_d = din("cosl", (128, SL))
    sinlsg_d = din("sinlsg", (128, SL))         # sign-folded sin
    cosf_d = din("cosf", (128, S))
    sinfsg_d = din("sinfsg", (128, S))
    wqb2_d = din("wqb2", (128, KQ * 768))       # [p, k*768 + col]
    kct2_d = din("kct2", (128, HPC * KC * 128))  # [p, (i*4+c)*128 + d]
    vcp2_d = din("vcp2", (128, KC * HPC * DV))  # [p, c*512 + col]
    wo2_d = din("wo2", (128, HPC * NW * 512))   # [p, (i*NW+n)*512 + col]
    steps_d = din("steps", (128, 4 * 512))      # keep01: [p*128+r <= q]
    out_d = nc.dram_tensor("out_partial", [S, HID], bf,
                           kind="ExternalOutput").ap()

    cc1_in = nc.dram_tensor("cc1_in", [KVLR + 128, SL], bf).ap()
    cc1_out = nc.dram_tensor("cc1_out", [NC_N * (KVLR + 128), SL], bf,
                             addr_space="Shared").ap()
    cc2_in = nc.dram_tensor("cc2_in", [QLR, SL], bf).ap()
    cc2_out = nc.dram_tensor("cc2_out", [NC_N * QLR, SL], bf,
                             addr_space="Shared").ap()

    with tile.TileContext(nc) as tc, ExitStack() as ctx:
        def pool(name, bufs):
            return ctx.enter_context(tc.tile_pool(name=name, bufs=bufs))

        p_hs = pool("hs", 4)
        p_w = pool("wstr", 3)
        p_raw = pool("raw", 12)
        p_sq = pool("sqt", 1)
        p_scn = pool("scn", 2)
        p_sml = pool("sml", 4)
        p_one = pool("ones", 2)
        p_cs = pool("cs", 2)
        p_csl = pool("csl", 2)
        p_bc = pool("bc", 2)
        p_kc = pool("kc", 1)
        p_vc = pool("vc", 1)
        p_wqb = pool("wqb", 2)
        p_wo = pool("wo", 5)
        p_msk = pool("msk", 1)
        p_kpeg = pool("kpeg", 4)
        p_K = pool("Kt", 4)
        p_V = pool("Vt", 16)
        p_qn = pool("Qn", 4)
        p_rope = pool("rope", 2)
        p_f32 = pool("fr32", 2)
        p_ckvg = pool("ckvg", 16)
        p_wk = pool("wk", 14)       # shared ring: qlg -> P tiles
        p_pacc = pool("pacc", 2)
        p_oT = pool("oT", 4)
        p_out = pool("outst", 2)

        pp_o = ctx.enter_context(
            tc.tile_pool(name="ppo", bufs=3, space="PSUM"))
        pp_s = ctx.enter_context(
            tc.tile_pool(name="pps", bufs=5, space="PSUM"))
        pp_sm = pp_s

        ones_col = p_one.tile([128, 1], bf)       # lhsT for column sums
        nc.vector.memset(ones_col[:], 1.0)
        eps_t = p_one.tile([1, 1], f32, tag="eps", name="eps")
        nc.vector.memset(eps_t[:], EPS)

        def rope_apply(dst, src_ps, cos_t, sin_t, c0, width, u1, u2):
            """dst(bf16) = src*cos + rot(src)*sin_signed, via partition-offset
            muls. src_ps is a [128, width] f32 PSUM pack of 64-dim halves."""
            nc.vector.tensor_mul(u1[:, :width], src_ps[:],
                                 cos_t[:, c0:c0 + width])
            for blk in (0, 64):
                nc.vector.tensor_mul(
                    u2[blk:blk + 32, :width],
                    src_ps[blk + 32:blk + 64, :],
                    sin_t[blk:blk + 32, c0:c0 + width])
                nc.vector.tensor_mul(
                    u2[blk + 32:blk + 64, :width],
                    src_ps[blk:blk + 32, :],
                    sin_t[blk + 32:blk + 64, c0:c0 + width])
            nc.vector.tensor_add(dst, u1[:, :width], u2[:, :width])

        # ---------------- FRONT 1: ckv + k_pe pass (then AG1) --------------
        ckv_ps = ([pp_o.tile([128, SL], f32, tag="po", name="po")
                   for _ in range(3)] +
                  [pp_s.tile([128, SL], f32, tag="ps", name="ps")])
        pe_ps = pp_s.tile([128, SL], f32, tag="ps", name="ps")
        for k2 in range(K2):
            hs2 = p_hs.tile([128, 2 * SL], bf, tag="hs", name="hs")
            nc.sync.dma_start(hs2[:], hsT2_d[:, k2 * 512:(k2 + 1) * 512])
            wv = p_w.tile([128, 1280], bf, tag="wkv", name="wkv", bufs=3)
            nc.scalar.dma_start(wv[:], wkvpe2_d[:, k2 * 1280:(k2 + 1) * 1280])
            for kk in range(2):
                k = 2 * k2 + kk
                hh = hs2[:, kk * SL:(kk + 1) * SL]
                wk0 = kk * 640
                for c in range(KC):
                    nc.tensor.matmul(ckv_ps[c][:],
                                     wv[:, wk0 + c * 128:wk0 + (c + 1) * 128],
                                     hh, start=(k == 0), stop=(k == KH - 1))
                nc.tensor.matmul(pe_ps[:], wv[:, wk0 + KVLR:wk0 + KVLR + 128],
                                 hh,
                                 start=(k == 0), stop=(k == KH - 1))
        ssq_kv = pp_sm.tile([1, SL], f32, tag="ps", name="ps")
        raw_kv = []
        for c in range(KC):
            r = p_raw.tile([128, SL], bf, tag="raw", name="raw")
            nc.scalar.activation(r[:], ckv_ps[c][:], AF.Copy)
            raw_kv.append(r)
            sq = p_sq.tile([128, SL], bf, tag="sq", name="sq")
            nc.scalar.activation(sq[:], ckv_ps[c][:], AF.Square)
            nc.tensor.matmul(ssq_kv[:], ones_col[:], sq[:],
                             start=(c == 0), stop=(c == KC - 1))
        t_kv = p_sml.tile([1, SL], f32, tag="sml", name="sml")
        nc.scalar.activation(t_kv[:], ssq_kv[:], AF.Sqrt,
                             bias=eps_t[:], scale=1.0 / KVLR)
        s_kv = p_sml.tile([1, SL], f32, tag="sml", name="sml")
        nc.vector.reciprocal(s_kv[:], t_kv[:])
        bkv_sb = p_bc.tile([128, 512], f32, tag="bc", name="bc")
        nc.gpsimd.partition_broadcast(bkv_sb[:, :SL], s_kv[:])
        for c in range(KC):
            cn = p_scn.tile([128, SL], bf, tag="scn", name="scn")
            nc.vector.tensor_mul(cn[:], raw_kv[c][:], bkv_sb[:, :SL])
            nc.scalar.dma_start(cc1_in[c * 128:(c + 1) * 128, :], cn[:])
        # rope k_pe (partition-offset rotation, sign folded into sin table)
        cosl_t = p_csl.tile([128, SL], bf, tag="csl", name="csl")
        sinl_t = p_csl.tile([128, SL], bf, tag="csl", name="csl")
        nc.sync.dma_start(cosl_t[:], cosl_d[:, :])
        nc.sync.dma_start(sinl_t[:], sinlsg_d[:, :])
        u1f = p_f32.tile([128, 512], f32, tag="f32", name="f32")
        u2f = p_f32.tile([128, 512], f32, tag="f32", name="f32")
        kpe_n = p_scn.tile([128, SL], bf, tag="scn", name="scn")
        rope_apply(kpe_n[:], pe_ps, cosl_t, sinl_t, 0, SL, u1f, u2f)
        nc.scalar.dma_start(cc1_in[KVLR:KVLR + 128, :], kpe_n[:])

        nc.gpsimd.collective_compute(
            "AllGather", mybir.AluOpType.bypass,
            ins=[cc1_in[:]], outs=[cc1_out[:]],
            replica_groups=[list(range(NC_N))],
        )

        # ---------------- FRONT 2: q_lora pass (then AG2) ------------------
        # resident back-end weights stream on the scalar/gpsimd rings
        kct_t = p_kc.tile([128, HPC * KC * 128], bf, tag="kc", name="kc")
        nc.scalar.dma_start(kct_t[:], kct2_d[:, :])
        vcp_t = p_vc.tile([128, KC * HPC * DV], bf, tag="vc", name="vc")
        nc.gpsimd.dma_start(vcp_t[:], vcp2_d[:, :])
        wqb_t = []
        for h in range(2):
            t = p_wqb.tile([128, 6 * 768], bf, tag="wqb", name="wqb")
            eng = nc.scalar if h == 0 else nc.gpsimd
            eng.dma_start(t[:], wqb2_d[:, h * 6 * 768:(h + 1) * 6 * 768])
            wqb_t.append(t)
        cosf_t = p_cs.tile([128, S], bf, tag="cs", name="cs")
        sinf_t = p_cs.tile([128, S], bf, tag="cs", name="cs")
        nc.scalar.dma_start(cosf_t[:], cosf_d[:, :])
        nc.gpsimd.dma_start(sinf_t[:], sinfsg_d[:, :])
        steps_t = p_msk.tile([128, 4 * 512], bf, tag="msk", name="msk")
        nc.scalar.dma_start(steps_t[:], steps_d[:, :])

        def wqb_ap(k, c0, c1):
            return wqb_t[k // 6][:, (k % 6) * 768 + c0:(k % 6) * 768 + c1]

        ssq_g = []
        raw_q = []
        for g in range(2):
            ql_ps = ([pp_o.tile([128, SL], f32, tag="po", name="po")
                      for _ in range(3)] +
                     [pp_s.tile([128, SL], f32, tag="ps", name="ps")
                      for _ in range(3)])
            for k2 in range(K2):
                hs2 = p_hs.tile([128, 2 * SL], bf, tag="hs", name="hs")
                heng = nc.sync if k2 % 2 == 0 else nc.scalar
                heng.dma_start(hs2[:], hsT2_d[:, k2 * 512:(k2 + 1) * 512])
                wq = []
                for kk in range(2):
                    k = 2 * k2 + kk
                    w = p_w.tile([128, QLR // 2], bf, tag="wqa", name="wqa",
                                 bufs=6)
                    eng = nc.sync if kk == 0 else nc.scalar
                    eng.dma_start(
                        w[:], wqa_d[k * 128:(k + 1) * 128,
                                    g * (QLR // 2):(g + 1) * (QLR // 2)])
                    wq.append(w)
                for kk in range(2):
                    k = 2 * k2 + kk
                    hh = hs2[:, kk * SL:(kk + 1) * SL]
                    for mi in range(6):
                        nc.tensor.matmul(ql_ps[mi][:],
                                         wq[kk][:, mi * 128:(mi + 1) * 128],
                                         hh, start=(k == 0),
                                         stop=(k == KH - 1))
            ssq_gt = pp_sm.tile([1, SL], f32, tag="ps", name="ps")
            ssq_g.append(ssq_gt)
            for mi in range(6):
                r = p_raw.tile([128, SL], bf, tag="raw", name="raw")
                nc.scalar.activation(r[:], ql_ps[mi][:], AF.Copy)
                raw_q.append(r)
                sq = p_sq.tile([128, SL], bf, tag="sq", name="sq")
                nc.scalar.activation(sq[:], ql_ps[mi][:], AF.Square)
                nc.tensor.matmul(ssq_gt[:], ones_col[:], sq[:],
                                 start=(mi == 0), stop=(mi == 5))
        ssg0 = p_sml.tile([1, SL], f32, tag="sml", name="sml")
        nc.scalar.activation(ssg0[:], ssq_g[0][:], AF.Copy)
        ssum_q = p_sml.tile([1, SL], f32, tag="sml", name="sml")
        nc.vector.tensor_add(ssum_q[:], ssg0[:], ssq_g[1][:])
        t_q = p_sml.tile([1, SL], f32, tag="sml", name="sml")
        nc.scalar.activation(t_q[:], ssum_q[:], AF.Sqrt,
                             bias=eps_t[:], scale=1.0 / QLR)
        s_q = p_sml.tile([1, SL], f32, tag="sml", name="sml")
        nc.vector.reciprocal(s_q[:], t_q[:])
        bq_sb = p_bc.tile([128, 512], f32, tag="bc", name="bc")
        nc.gpsimd.partition_broadcast(bq_sb[:, :SL], s_q[:])
        for m in range(KQ):
            qn = p_scn.tile([128, SL], bf, tag="scn", name="scn")
            nc.vector.tensor_mul(qn[:], raw_q[m][:], bq_sb[:, :SL])
            nc.scalar.dma_start(cc2_in[m * 128:(m + 1) * 128, :], qn[:])

        nc.gpsimd.collective_compute(
            "AllGather", mybir.AluOpType.bypass,
            ins=[cc2_in[:]], outs=[cc2_out[:]],
            replica_groups=[list(range(NC_N))],
        )

        # ---------------- BACK: K/V expansion (needs AG1 only) -------------
        RPC = 512 // SL     # AG rank-blocks per 512-wide seq chunk
        g_engs = [nc.scalar, nc.gpsimd, nc.sync]
        g_rr = [0]

        def rr_eng():
            e = g_engs[g_rr[0] % 3]
            g_rr[0] += 1
            return e

        ckvg = {}
        for c in range(KC):
            for sc in range(SC):
                t = p_ckvg.tile([128, 512], bf, tag="ckvg", name="ckvg")
                for half in range(RPC):
                    r = RPC * sc + half
                    rr_eng().dma_start(
                        t[:, half * SL:(half + 1) * SL],
                        cc1_out[r * (KVLR + 128) + c * 128:
                                r * (KVLR + 128) + (c + 1) * 128, :])
                ckvg[(c, sc)] = t
        kpeg = {}
        for sc in range(SC):
            t = p_kpeg.tile([128, 512], bf, tag="kpeg", name="kpeg")
            for half in range(RPC):
                r = RPC * sc + half
                rr_eng().dma_start(
                    t[:, half * SL:(half + 1) * SL],
                    cc1_out[r * (KVLR + 128) + KVLR:
                            r * (KVLR + 128) + KVLR + 128, :])
            kpeg[sc] = t
        # w_o resident tiles: streamed after the gathers
        wo_t = []
        for h in range(5):
            t = p_wo.tile([128, 8 * 512], bf, tag="wo", name="wo")
            eng = g_engs[h % 2]
            eng.dma_start(t[:], wo2_d[:, h * 8 * 512:(h + 1) * 8 * 512])
            wo_t.append(t)

        def wo_ap(i, n):
            b = i * NW + n
            return wo_t[b // 8][:, (b % 8) * 512:(b % 8 + 1) * 512]

        # K^T per head: [DN, S]
        K_t = []
        for i in range(HPC):
            kt = p_K.tile([128, S], bf, tag="K", name="K")
            K_t.append(kt)
            for sc in range(SC):
                ps = pp_s.tile([128, 512], f32, tag="ps", name="ps")
                for c in range(KC):
                    nc.tensor.matmul(
                        ps[:],
                        kct_t[:, (i * KC + c) * 128:(i * KC + c + 1) * 128],
                        ckvg[(c, sc)][:],
                        start=(c == 0), stop=(c == KC - 1))
                nc.scalar.activation(kt[:, sc * 512:(sc + 1) * 512], ps[:],
                                     AF.Copy)

        # V natural: per seq-block [128, 4*DV]
        V_t = []
        for sb in range(SB):
            ps = pp_s.tile([128, 512], f32, tag="ps", name="ps")
            for c in range(KC):
                nc.tensor.matmul(
                    ps[:],
                    ckvg[(c, sb // 4)][:, (sb % 4) * 128:(sb % 4 + 1) * 128],
                    vcp_t[:, c * 512:(c + 1) * 512],
                    start=(c == 0), stop=(c == KC - 1))
            vt = p_V.tile([128, HPC * DV], bf, tag="V", name="V")
            nc.scalar.activation(vt[:], ps[:], AF.Copy)
            V_t.append(vt)

        # ---------------- BACK: per-chunk Q proj + attention + w_o ---------
        qg_engs = [nc.gpsimd, nc.scalar]

        def gather_qlg(sc):
            qlg = []
            for k in range(KQ):
                t = p_wk.tile([128, 512], bf, tag="wk", name="wk")
                for half in range(RPC):
                    r = RPC * sc + half
                    qg_engs[(2 * k + half) % 2].dma_start(
                        t[:, half * SL:(half + 1) * SL],
                        cc2_out[r * QLR + k * 128:r * QLR + (k + 1) * 128, :])
                qlg.append(t)
            return qlg

        qlg = gather_qlg(0)
        for sc in range(SC):
            # --- Q^T nope per head + pe packs (roped) ---
            qn_t = []
            for i in range(HPC):
                ps = pp_o.tile([128, 512], f32, tag="po", name="po")
                for k in range(KQ):
                    nc.tensor.matmul(ps[:], wqb_ap(k, i * 128, (i + 1) * 128),
                                     qlg[k][:], start=(k == 0),
                                     stop=(k == KQ - 1))
                qt = p_qn.tile([128, 512], bf, tag="Qn", name="Qn")
                nc.scalar.activation(qt[:], ps[:], AF.Copy)
                qn_t.append(qt)
            roped = []
            for pkt in range(2):
                ps_pe = pp_s.tile([128, 512], f32, tag="ps", name="ps")
                for k in range(KQ):
                    nc.tensor.matmul(
                        ps_pe[:],
                        wqb_ap(k, 512 + pkt * 128, 512 + (pkt + 1) * 128),
                        qlg[k][:], start=(k == 0), stop=(k == KQ - 1))
                u1 = p_f32.tile([128, 512], f32, tag="f32", name="f32")
                u2 = p_f32.tile([128, 512], f32, tag="f32", name="f32")
                rp = p_rope.tile([128, 512], bf, tag="rope", name="rope")
                rope_apply(rp[:], ps_pe, cosf_t, sinf_t, sc * 512, 512, u1, u2)
                roped.append(rp)

            # --- attention: score stream runs ahead; o/pacc consumes trail
            # by LAG iterations (cross-head), so the exp -> attn@v serial
            # chain is amortized over LAG j-steps and the PE stays busy ---
            LAG = 9
            nj = 4 * sc + 4
            o_ps_l, oT = [], [None] * HPC
            pacc_l, pts_l = [], []
            pending = []

            def consume_one():
                ci, cj = pending.pop(0)
                nc.tensor.matmul(o_ps_l[ci][:],
                                 V_t[cj][:, ci * DV:(ci + 1) * DV],
                                 pts_l[ci][cj][:], start=(cj == 0),
                                 stop=(cj == nj - 1))
                if cj == 0:
                    nc.vector.tensor_copy(pacc_l[ci][:], pts_l[ci][cj][:])
                else:
                    nc.vector.tensor_add(pacc_l[ci][:], pacc_l[ci][:],
                                         pts_l[ci][cj][:])
                if cj == nj - 1:
                    d_ps = pp_sm.tile([1, 512], f32, tag="ps", name="ps")
                    nc.tensor.matmul(d_ps[:], ones_col[:], pacc_l[ci][:],
                                     start=True, stop=True)
                    dinv = p_sml.tile([1, 512], f32, tag="sml", name="sml")
                    nc.vector.reciprocal_approx_fast(dinv[:], d_ps[:])
                    bc_sb = p_bc.tile([128, 512], f32, tag="bc", name="bc")
                    nc.gpsimd.partition_broadcast(bc_sb[:], dinv[:])
                    ot = p_oT.tile([128, 512], bf, tag="oT", name="oT")
                    nc.vector.tensor_mul(ot[:], o_ps_l[ci][:], bc_sb[:])
                    oT[ci] = ot

            for i in range(HPC):
                pkt, hp = i // 2, i % 2
                o_ps = pp_o.tile([128, 512], f32, tag="po", name="po")
                o_ps_l.append(o_ps)
                pacc = p_pacc.tile([128, 512], bf, tag="pacc", name="pacc")
                pacc_l.append(pacc)
                pts_l.append([])
                for j in range(nj):
                    s_ps = pp_s.tile([128, 512], f32, tag="ps", name="ps")
                    nc.tensor.matmul(s_ps[:],
                                     K_t[i][:, j * 128:(j + 1) * 128],
                                     qn_t[i][:], start=True, stop=False)
                    nc.tensor.matmul(
                        s_ps[:],
                        kpeg[j // 4][hp * 64:(hp + 1) * 64,
                                     (j % 4) * 128:(j % 4 + 1) * 128],
                        roped[pkt][hp * 64:(hp + 1) * 64, :],
                        start=False, stop=True)
                    if len(pending) >= LAG:
                        consume_one()
                    if j >= 4 * sc:
                        # masked diagonal: exp then 0/1 keep-multiply on the
                        # vector engine — with the LAG consume queue this
                        # hop has ~LAG iterations of slack off the PE chain
                        p = j - 4 * sc
                        pr = p_wk.tile([128, 512], bf, tag="wk", name="wk")
                        nc.scalar.activation(pr[:], s_ps[:], AF.Exp)
                        pt = p_wk.tile([128, 512], bf, tag="wk", name="wk")
                        nc.vector.tensor_mul(pt[:], pr[:],
                                             steps_t[:, p * 512:(p + 1) * 512])
                    else:
                        pt = p_wk.tile([128, 512], bf, tag="wk", name="wk")
                        nc.scalar.activation(pt[:], s_ps[:], AF.Exp)
                    pts_l[i].append(pt)
                    pending.append((i, j))
            while pending:
                consume_one()

            # prefetch next chunk's gathered q_lora during w_o
            if sc + 1 < SC:
                qlg = gather_qlg(sc + 1)

            # --- w_o partial for this seq chunk ---
            # sbl outer / paired n inner: two [128,512] psum groups stage
            # into one [128,1024] tile and ship with a single DMA
            for sbl in range(4):
                sb = sc * 4 + sbl
                for n2 in range(NW // 2):
                    ob = p_out.tile([128, 1024], bf, tag="outst",
                                    name="outst")
                    for half in range(2):
                        n = 2 * n2 + half
                        ps = pp_s.tile([128, 512], f32, tag="ps", name="ps")
                        for i in range(HPC):
                            nc.tensor.matmul(
                                ps[:], oT[i][:, sbl * 128:(sbl + 1) * 128],
                                wo_ap(i, n), start=(i == 0),
                                stop=(i == HPC - 1))
                        ceng = nc.scalar if half == 0 else nc.vector
                        if half == 0:
                            ceng.activation(ob[:, :512], ps[:], AF.Copy)
                        else:
                            ceng.tensor_copy(ob[:, 512:], ps[:])
                    nc.sync.dma_start(
                        out_d[sb * 128:(sb + 1) * 128,
                              n2 * 1024:(n2 + 1) * 1024], ob[:])

    nc.compile()
    return nc


def _prep_inputs(inputs):
    """Host-side sharding + weight folding. Returns in_maps (list of 8 dicts)."""
    BF = _bf16()

    hs = np.asarray(inputs['hidden_states'], np.float32)
    pos = np.asarray(inputs['positions'])
    w_qa = np.asarray(inputs['w_qa'], np.float32)
    q_a_ln_w = np.asarray(inputs['q_a_ln_w'], np.float32)
    w_qb = np.asarray(inputs['w_qb'], np.float32)
    w_kva = np.asarray(inputs['w_kva'], np.float32)
    kv_a_ln_w = np.asarray(inputs['kv_a_ln_w'], np.float32)
    kc = np.asarray(inputs['kc'], np.float32)
    vc = np.asarray(inputs['vc'], np.float32)
    w_o = np.asarray(inputs['w_o'], np.float32)

    perm = np.concatenate([np.arange(0, DR, 2), np.arange(1, DR, 2)])
    inv_freq = 1.0 / (ROPE_BASE ** (np.arange(0, DR, 2, dtype=np.float64) / DR))
    freqs = pos.astype(np.float64)[None, :] * inv_freq[:, None]     # [32, S]
    cosT = np.cos(freqs).astype(np.float32)
    sinT = np.sin(freqs).astype(np.float32)
    cos128 = np.tile(cosT, (4, 1)).astype(BF)                        # [128, S]
    sin128 = np.tile(sinT, (4, 1)).astype(np.float32)
    sgn = np.where((np.arange(128) % 64) < 32, -1.0, 1.0)[:, None]
    sinsg128 = (sin128 * sgn).astype(BF)

    scale = DQ ** -0.5
    w_qb_eff = ((w_qb * q_a_ln_w[:, None]) * scale).reshape(QLR, H, DQ)

    w_pe = w_kva[:, KVLR:][:, perm]
    wkvpe = np.concatenate([w_kva[:, :KVLR], w_pe, w_pe], 1)   # [HID, 640]
    K2h = (HID // 128) // 2
    wkvpe2 = wkvpe.reshape(K2h, 2, 128, 640).transpose(2, 0, 1, 3) \
        .reshape(128, K2h * 1280).astype(BF)

    kc_f = kc * kv_a_ln_w[None, None, :]
    vc_f = vc * kv_a_ln_w[None, :, None]

    # keep tables: steps[r, p*512+q] = 0 if p*128+r > q else 1 (causal keep)
    steps = np.zeros((128, 4 * 512), np.float32)
    rr = np.arange(128)[:, None]
    qq = np.arange(512)[None, :]
    for p in range(4):
        steps[:, p * 512:(p + 1) * 512] = (p * 128 + rr <= qq)
    steps_b = steps.astype(BF)

    wqa_b = w_qa.astype(BF)

    K2 = (HID // 128) // 2
    NW = HID // 512

    in_maps = []
    for core in range(NC_N):
        rows = slice(core * SL, (core + 1) * SL)
        h0 = core * HPC

        hsT = np.ascontiguousarray(hs[rows].T)                   # [HID, SL]
        hsT2 = hsT.reshape(K2, 2, 128, SL).transpose(2, 0, 1, 3) \
            .reshape(128, K2 * 2 * SL)

        wqb_all = np.empty((QLR, 768), np.float32)
        for i in range(HPC):
            wqb_all[:, i * 128:(i + 1) * 128] = w_qb_eff[:, h0 + i, :DN]
        for pkt in range(2):
            a, b = h0 + 2 * pkt, h0 + 2 * pkt + 1
            pe_a = w_qb_eff[:, a, DN:][:, perm]
            pe_b = w_qb_eff[:, b, DN:][:, perm]
            wqb_all[:, 512 + pkt * 128:512 + pkt * 128 + 64] = pe_a
            wqb_all[:, 512 + pkt * 128 + 64:512 + (pkt + 1) * 128] = pe_b
        wqb2 = wqb_all.reshape(12, 128, 768).transpose(1, 0, 2) \
            .reshape(128, 12 * 768)

        kct = np.stack([kc_f[h0 + i].T[c * 128:(c + 1) * 128]
                        for i in range(HPC) for c in range(KVLR // 128)])
        kct2 = kct.transpose(1, 0, 2).reshape(128, -1)           # [128, 2048]

        vcp = np.concatenate([vc_f[h0 + i] for i in range(HPC)], 1)
        vcp2 = vcp.reshape(KVLR // 128, 128, HPC * DV) \
            .transpose(1, 0, 2).reshape(128, -1)                 # [128, 2048]

        wo_sh = w_o[h0 * DV:(h0 + HPC) * DV, :]                  # [512, HID]
        wo2 = wo_sh.reshape(HPC, 128, NW, 512).transpose(0, 2, 1, 3) \
            .reshape(HPC * NW, 128, 512).transpose(1, 0, 2) \
            .reshape(128, -1)                                    # [128, 20480]

        in_maps.append({
            "hsT2": hsT2.astype(BF),
            "wqa": wqa_b,
            "wkvpe2": wkvpe2,
            "cosl": np.ascontiguousarray(cos128[:, rows]),
            "sinlsg": np.ascontiguousarray(sinsg128[:, rows]),
            "cosf": cos128,
            "sinfsg": sinsg128,
            "wqb2": wqb2.astype(BF),
            "kct2": kct2.astype(BF),
            "vcp2": vcp2.astype(BF),
            "wo2": wo2.astype(BF),
            "steps": steps_b,
        })
    return in_maps


def _get_program():
    if "nc" not in _CACHE:
        _CACHE["nc"] = _build_program()
    return _CACHE["nc"]


def run(inputs, trace=False, trace_kwargs=None):
    """Build (cached), run on 8 cores, return (output, BassKernelResults)."""
    from concourse.bass_utils import run_bass_kernel_spmd

    nc = _get_program()
    in_maps = _prep_inputs(inputs)
    res = run_bass_kernel_spmd(nc, in_maps, list(range(NC_N)),
                               trace=trace, **(trace_kwargs or {}))
    out = np.zeros((S, HID), np.float32)
    for r in res.results:
        out += np.asarray(r["out_partial"], dtype=np.float32)
    return out, res


def kernel(**inputs) -> np.ndarray:
    out, _ = run(inputs, trace=False)
    return out
